# revision 54
# baseline (speedup 1.0000x reference)
"""GCN classifier kernel for Trainium2, 8 NeuronCores.

Strategy: graph-aligned node sharding (64 graphs/core), padded to NPC nodes.
Edges bucketed by (dst-block-of-128, src-quarter) cells; per-edge messages
are fetched from an AllGather-replicated bf16 node table with SWDGE
dma_gather using merged 1024-idx s-major streams, then aggregated per dst
block with one-hot matmuls whose lhsT is an ew-weighted mask:

  agg[dst_blk] = sum_chunks maskW[:, :, c]^T @ msg[chunk]   (+ identity
  self-loop from the resident slab)

maskW is laid out [128 edge, 128 dst, C] with the chunk dim innermost so
both mask-build ops (is_equal vs dstloc, mult by ew) are packed-innermost
tensor_tensor ops that hit the DVE 2x perf mode; the matmul reads lhsT with
a strided AP. The ew multiply rides the aggregation matmul for free.

PSUM drains run on the scalar (ACT) engine (DVE PSUM reads are slow).
AllGather is split in two halves (half-major table layout) to overlap the
collective with the second half of each layer. Mean pooling is a one-hot
matmul with 1/count baked into the selector; max pooling keeps the gather
path. LayerNorm/residual/ReLU epilogues run per 128-node block on ACT/DVE.
"""

import sys
import types

sys.path.insert(0, "/opt/trn_rl_repo")

import numpy as np
import ml_dtypes

BF16 = ml_dtypes.bfloat16

# Shim antenv.axon_hooks (missing in this image) so trace=True can work.
try:
    import antenv.axon_hooks  # noqa: F401
except ImportError:
    try:
        from trn_agent_boot.trn_boot import _ntff_profile_via_ctypes
        _hook = _ntff_profile_via_ctypes('/opt/axon/libaxon_pjrt.so')
    except Exception:
        _hook = None
    _mod = types.ModuleType('antenv.axon_hooks')
    _mod.get_axon_ntff_profile_hook = lambda: _hook
    sys.modules['antenv.axon_hooks'] = _mod

import concourse.bacc as bacc
import concourse.mybir as mybir
import concourse.tile as tile
import concourse.bass_utils as bass_utils

# No bucket access in this container.
bass_utils.upload_artifacts = lambda tmpdir: tmpdir

F32 = mybir.dt.float32
BF = mybir.dt.bfloat16
I16 = mybir.dt.int16
AOP = mybir.AluOpType
ACTF = mybir.ActivationFunctionType
AXX = mybir.AxisListType.X

NCORES = 8
H = 128        # hidden channels
IN = 96        # in channels
ED = 8         # edge dim
NCLS = 100     # classes
L = 4          # layers
NGRAPH = 512   # graphs
GPC = NGRAPH // NCORES
EPS_LN = 1e-5
MAXG = 1024    # dma_gather num_idxs hard limit (2048 wedges the device)

import os
DBG_NOGATHER = os.environ.get("K_NOGATHER") == "1"
DBG_NOMAXPOOL = os.environ.get("K_NOMAXPOOL") == "1"
DBG_NOGMEAN = os.environ.get("K_NOGMEAN") == "1"
DBG_NOMASK = os.environ.get("K_NOMASK") == "1"


def _split_waits(nc, max_waits=1):
    """This container's walrus rejects >1 sync wait per instruction; move
    extra waits onto preceding NOPs on the same engine."""
    n = 0
    for f in nc.m.functions:
        for bb in f.blocks:
            new_list = []
            for ins in bb.instructions:
                si = ins.sync_info
                if si and si.on_wait and len(si.on_wait) > max_waits:
                    waits = list(si.on_wait)
                    extra, keep = waits[:-max_waits], waits[-max_waits:]
                    for i, w in enumerate(extra):
                        nop = mybir.InstNoOp(name=f"{ins.name}-ws{i}", ins=[], outs=[])
                        nop.engine = ins.engine
                        nop.sync_info = mybir.SyncInfo(on_wait=[w], on_update=[])
                        new_list.append(nop)
                        n += 1
                    si.on_wait = keep
                new_list.append(ins)
            bb.instructions[:] = new_list
    return n


def _fix_act_tables(nc, set_id=6):
    """All activation funcs used here live in act table 6
    (natural_log_exp_and_others); the greedy per-func chooser ping-pongs
    between tables 0/5 costing ~27us per reload. Unify and dedupe."""
    removed = 0
    for f in nc.m.functions:
        for bb in f.blocks:
            new_list = []
            loaded = False
            for ins in bb.instructions:
                if isinstance(ins, mybir.InstLoadActFuncSet):
                    ins.act_func_set_id = set_id
                    si = ins.sync_info
                    has_sync = si and (si.on_wait or si.on_update)
                    if loaded and not has_sync:
                        removed += 1
                        continue
                    if loaded and has_sync:
                        nop = mybir.InstNoOp(name=ins.name + "-actdedup", ins=[], outs=[])
                        nop.engine = ins.engine
                        nop.sync_info = si
                        new_list.append(nop)
                        removed += 1
                        continue
                    loaded = True
                new_list.append(ins)
            bb.instructions[:] = new_list
    return removed


def _ru(x, m):
    return (x + m - 1) // m * m


def _wrap_idxs(idx):
    """[n] int -> [128, n//16] int16 SBUF wrap (i -> partition i%16, col i//16),
    replicated over the 8 gpsimd cores."""
    n = len(idx)
    assert n % 16 == 0
    a = np.asarray(idx, np.int16).reshape(n // 16, 16).T.copy()
    return np.tile(a, (8, 1))


class Plan:
    pass


def make_plan(x, edge_index, batch, edge_attr):
    N = x.shape[0]
    E = edge_index.shape[1]
    p = Plan()
    p.N, p.E = N, E

    batch = np.asarray(batch, np.int64)
    src = np.asarray(edge_index[0], np.int64)
    dst = np.asarray(edge_index[1], np.int64)

    node_start = np.searchsorted(batch, np.arange(NGRAPH + 1))  # [513]
    core_start = node_start[::GPC][:NCORES].astype(np.int64)
    core_end = np.append(core_start[1:], N).astype(np.int64)
    core_cnt = core_end - core_start
    NPC = max(512, _ru(int(core_cnt.max()), 128))
    NBLK = NPC // 128
    NPAD = NCORES * NPC
    # Four gather windows (int16 idx => window rows <= 32767), uneven: the
    # last window is small so its AllGather (the only exposed one at layer
    # boundaries) is short.
    MAXWB = 32767 // (NCORES * 128)  # max window size in blocks (31)
    base = max(1, min(MAXWB, round(NBLK * 0.26)))
    wblk = [base, base, base, NBLK - 3 * base]
    assert 1 <= wblk[3] <= MAXWB, (NBLK, wblk)
    woff = np.concatenate([[0], np.cumsum(wblk)]).astype(np.int64)
    p.wblk, p.woff = wblk, woff
    p.NPC, p.NBLK, p.NPAD = NPC, NBLK, NPAD
    p.core_start, p.core_cnt = core_start, core_cnt

    owner = np.searchsorted(core_start, np.arange(N), side='right') - 1
    loc = np.arange(N) - core_start[owner]
    # window-major table row: table window w holds every core's local node
    # blocks [woff[w], woff[w+1]), and equals gather window w.
    blk_of = loc >> 7
    wsel = np.searchsorted(woff, blk_of, side='right') - 1
    table_row = (woff[wsel] * NCORES * 128 + owner * (woff[wsel + 1] - woff[wsel]) * 128
                 + (loc - woff[wsel] * 128))

    src_t_all = table_row[src]
    srange_all = wsel[src]
    d_owner = owner[dst]
    d_loc = loc[dst]
    blk_all = d_loc >> 7
    dloc_all = d_loc & 127
    cell_all = blk_all * 4 + srange_all  # b-major cell id
    NCELL = NBLK * 4

    order = np.lexsort((src_t_all, cell_all, d_owner))
    src_t = src_t_all[order]
    dloc = dloc_all[order]
    cell = cell_all[order]
    e_owner = d_owner[order]
    ea_perm = np.asarray(edge_attr, np.float32)[order]

    counts = np.zeros((NCORES, NCELL), np.int64)
    for r in range(NCORES):
        m = e_owner == r
        counts[r] = np.bincount(cell[m], minlength=NCELL)
    core_off = np.searchsorted(e_owner, np.arange(NCORES + 1))
    core_cell_off = np.zeros((NCORES, NCELL + 1), np.int64)
    for r in range(NCORES):
        core_cell_off[r, 0] = core_off[r]
        core_cell_off[r, 1:] = np.cumsum(counts[r]) + core_off[r]

    cnum = _ru(counts.max(axis=0), 128)     # padded idx count per cell
    Cg = cnum // 128                        # chunks per cell
    p.Cg = Cg

    # b-major chunk columns
    cell_dl = np.concatenate([[0], np.cumsum(Cg)]).astype(np.int64)
    NDL = int(cell_dl[-1])
    p.NDL = NDL
    C_b = np.array([int(cell_dl[b * 4 + 4] - cell_dl[b * 4]) for b in range(NBLK)])
    p.C_b = C_b
    p.Cmax = int(C_b.max())
    assert p.Cmax >= 2

    # s-major gather streams: stream s = concat over b of cell (b, s) chunks
    stream_cells = [[b * 4 + s for b in range(NBLK)] for s in range(4)]
    stream_len = [int(sum(Cg[c] for c in cs)) for cs in stream_cells]  # chunks
    G_s = [(sl * 128 + MAXG - 1) // MAXG for sl in stream_len]
    p.G_s = G_s
    gbase = np.concatenate([[0], np.cumsum(G_s)]).astype(np.int64)
    p.NGATH = int(gbase[-1])

    # chunk (b-major col j) -> (gather g, slot) and per-(b,s) gather needs
    chunk_gather = np.zeros(NDL, np.int64)
    chunk_slot = np.zeros(NDL, np.int64)
    need_g = np.zeros((NBLK, 4), np.int64)   # gathers of stream s needed
    pos_s = [0, 0, 0, 0]
    for b in range(NBLK):
        for s in range(4):
            c = b * 4 + s
            for k in range(Cg[c]):
                j = cell_dl[c] + k
                pos = pos_s[s]
                chunk_gather[j] = gbase[s] + pos // 8
                chunk_slot[j] = pos % 8
                pos_s[s] += 1
            need_g[b, s] = (pos_s[s] + 7) // 8  # ceil chunks/8 so far
    p.chunk_gather, p.chunk_slot, p.need_g = chunk_gather, chunk_slot, need_g

    # fill per-core data
    dstloc_f = np.full((NCORES, 128, NDL), 255.0, np.float32)
    p.eaT_all = np.zeros((NCORES, 8, NDL * 128), BF16)
    idx_stream = np.zeros((NCORES, 4, max(G_s) * MAXG), np.int64)
    for r in range(NCORES):
        spos = [0, 0, 0, 0]
        for b in range(NBLK):
            for s in range(4):
                c = b * 4 + s
                if Cg[c] == 0:
                    continue
                a0 = core_cell_off[r, c]
                a1 = core_cell_off[r, c + 1]
                n_real = int(a1 - a0)
                num = int(cnum[c])
                # idxs for this cell (pad slots -> 0)
                iv = np.zeros(num, np.int64)
                if n_real:
                    iv[:n_real] = src_t[a0:a1] - int(woff[s]) * NCORES * 128
                idx_stream[r, s, spos[s]:spos[s] + num] = iv
                spos[s] += num
                # dstloc cols (b-major)
                dl = np.full(num, 255.0, np.float32)
                if n_real:
                    dl[:n_real] = dloc[a0:a1].astype(np.float32)
                dstloc_f[r, :, cell_dl[c]:cell_dl[c + 1]] = \
                    dl.reshape(Cg[c], 128).T
                # edge attrs (b-major)
                if n_real:
                    ea = np.zeros((num, ED), np.float32)
                    ea[:n_real] = ea_perm[a0:a1]
                    p.eaT_all[r, :, cell_dl[c] * 128:cell_dl[c + 1] * 128] = \
                        ea.T.astype(BF16)
    p.dstloc_all = dstloc_f.astype(BF16)

    # wrap idx streams into gather-major int16 [128, NGATH*64]
    p.idx_all = np.zeros((NCORES, 128, p.NGATH * (MAXG // 16)), np.int16)
    for r in range(NCORES):
        for s in range(4):
            for g in range(G_s[s]):
                iv = idx_stream[r, s, g * MAXG:(g + 1) * MAXG]
                gg = int(gbase[s]) + g
                p.idx_all[r, :, gg * 64:(gg + 1) * 64] = _wrap_idxs(iv)

    # x slab, transposed [96, NPC] per core
    p.xT = np.zeros((NCORES, IN, NPC), np.float32)
    xf = np.asarray(x, np.float32)
    for r in range(NCORES):
        p.xT[r, :, :core_cnt[r]] = xf[core_start[r]:core_end[r]].T

    # mean pooling selector: gsel[node p of block b, g] = 1/count(g)
    gcnt = (node_start[1:] - node_start[:-1]).astype(np.int64)
    p.gsel = np.zeros((NCORES, 128, NBLK * GPC), np.float32)
    for r in range(NCORES):
        for lid in range(int(core_cnt[r])):
            g = int(batch[core_start[r] + lid])
            jl = g - r * GPC
            b, pp = lid >> 7, lid & 127
            p.gsel[r, pp, b * GPC + jl] = 1.0 / max(int(gcnt[g]), 1)
    p.gsel = p.gsel.astype(BF16)

    # max pooling: gather idx per graph padded to MAXN (repeat first node)
    MAXN = max(128, _ru(int(gcnt.max()), 128))
    assert MAXN <= MAXG
    p.MAXN = MAXN
    ZROW = NPC
    p.pmax_idx = np.zeros((NCORES, 128, GPC * MAXN // 16), np.int16)
    for r in range(NCORES):
        mi = []
        for j in range(GPC):
            gid = r * GPC + j
            a = int(node_start[gid] - core_start[r])
            n = int(gcnt[gid])
            ids = np.arange(a, a + n)
            pad = MAXN - n
            mi.append(np.concatenate([ids, np.full(pad, ids[0] if n else ZROW)]))
        p.pmax_idx[r] = _wrap_idxs(np.concatenate(mi))
    # block (uniform across cores) after which max-gather group j0 can fire
    GPCALL = max(1, MAXG // MAXN)
    p.pool_grp_blk = np.zeros(GPC, np.int64)
    for j0 in range(0, GPC, GPCALL):
        j1 = min(j0 + GPCALL, GPC)
        endmax = 0
        for r in range(NCORES):
            e = int(node_start[min(r * GPC + j1, NGRAPH)] - core_start[r])
            endmax = max(endmax, (e + 127) // 128)
        p.pool_grp_blk[j0] = min(endmax, NBLK) - 1

    # iota3 [128, 128, Cmax] bf16: value d at (p, d, c)
    p.iota3 = np.tile(
        np.arange(128, dtype=np.float32)[None, :, None],
        (128, 1, p.Cmax)).reshape(128, 128 * p.Cmax).astype(BF16)
    return p


def build_nc(p, w):
    nc = bacc.Bacc("TRN2", num_devices=NCORES, detect_race_conditions=False,
                   num_swdge_queues=4)
    NPC, NBLK, NPAD = p.NPC, p.NBLK, p.NPAD
    wblk, woff = p.wblk, p.woff
    NDL, Cmax, C_b, Cg = p.NDL, p.Cmax, p.C_b, p.Cg
    cell_dl = np.concatenate([[0], np.cumsum(Cg)]).astype(np.int64)

    # ---- I/O ----
    t_xT = nc.dram_tensor("xT", [IN, NPC], F32, kind="ExternalInput")
    t_idx = nc.dram_tensor("idx", [128, p.NGATH * 64], I16, kind="ExternalInput")
    t_dstloc = nc.dram_tensor("dstloc", [128, NDL], BF, kind="ExternalInput")
    t_eaT = nc.dram_tensor("eaT", [8, NDL * 128], BF, kind="ExternalInput")
    t_iota3 = nc.dram_tensor("iota3", [128, 128 * Cmax], BF, kind="ExternalInput")
    t_gsel = nc.dram_tensor("gsel", [128, NBLK * GPC], BF, kind="ExternalInput")
    t_pmax_idx = nc.dram_tensor("pmax_idx", [128, GPC * p.MAXN // 16], I16,
                                kind="ExternalInput")
    t_W0 = nc.dram_tensor("W0", [IN, H], F32, kind="ExternalInput")
    t_resW = nc.dram_tensor("resW", [IN, H], F32, kind="ExternalInput")
    t_Wk = nc.dram_tensor("Wk", [H, 3 * H], F32, kind="ExternalInput")
    t_rows = nc.dram_tensor("rows", [128, 16 * H], F32, kind="ExternalInput")
    t_eeW1 = nc.dram_tensor("eeW1", [ED, H], BF, kind="ExternalInput")
    t_eeW2 = nc.dram_tensor("eeW2", [H, 1], BF, kind="ExternalInput")
    t_eeb1 = nc.dram_tensor("eeb1", [H, 1], F32, kind="ExternalInput")
    t_hW1 = nc.dram_tensor("hW1", [H, 2 * H], F32, kind="ExternalInput")
    t_hW2 = nc.dram_tensor("hW2", [H, NCLS], F32, kind="ExternalInput")
    t_ident_bf = nc.dram_tensor("ident_bf", [128, 128], BF, kind="ExternalInput")
    t_ident_f = nc.dram_tensor("ident_f", [128, 128], F32, kind="ExternalInput")
    t_out = nc.dram_tensor("out", [NGRAPH, NCLS], F32, kind="ExternalOutput")

    ee_b2 = float(np.asarray(w['ee_b2']).reshape(-1)[0])

    gq_counter = [0]

    def next_q():
        q = gq_counter[0] % 4
        gq_counter[0] += 1
        return q

    with tile.TileContext(nc) as tc:
        with (
            tc.tile_pool(name="const", bufs=1) as cp,
            tc.tile_pool(name="dram", bufs=1, space="DRAM") as dp,
            tc.tile_pool(name="ea", bufs=2) as eap,
            tc.tile_pool(name="eh", bufs=2) as ehp,
            tc.tile_pool(name="msg", bufs=13) as msgp,
            tc.tile_pool(name="mask", bufs=3) as maskp,
            tc.tile_pool(name="blk", bufs=3) as bp,
            tc.tile_pool(name="pag", bufs=2, space="PSUM") as pag,
            tc.tile_pool(name="pgm", bufs=1, space="PSUM") as pgm,
            tc.tile_pool(name="pmm", bufs=3, space="PSUM") as pmm,
            tc.tile_pool(name="psm", bufs=2, space="PSUM") as psm,
        ):
            # ---------- resident tiles ----------
            def load_const(t, shape, dtype, tag):
                tl = cp.tile(shape, dtype, tag=tag)
                nc.sync.dma_start(tl[:], t[:])
                return tl

            idx_sb = load_const(t_idx, [128, p.NGATH * 64], I16, "idx_sb")
            dstloc = load_const(t_dstloc, [128, NDL], BF, "dstloc")
            iota3 = load_const(t_iota3, [128, 128 * Cmax], BF, "iota3")
            gsel = load_const(t_gsel, [128, NBLK * GPC], BF, "gsel")
            W0 = load_const(t_W0, [IN, H], F32, "W0")
            resW = load_const(t_resW, [IN, H], F32, "resW")
            Wk = load_const(t_Wk, [H, 3 * H], F32, "Wk")
            rows = load_const(t_rows, [128, 16 * H], F32, "rows")
            eeW1 = load_const(t_eeW1, [ED, H], BF, "eeW1")
            eeW2 = load_const(t_eeW2, [H, 1], BF, "eeW2")
            eeb1 = load_const(t_eeb1, [H, 1], F32, "eeb1")
            hW1 = load_const(t_hW1, [H, 2 * H], F32, "hW1")
            hW2 = load_const(t_hW2, [H, NCLS], F32, "hW2")
            ident_bf = load_const(t_ident_bf, [128, 128], BF, "ident_bf")
            ident_f = load_const(t_ident_f, [128, 128], F32, "ident_f")
            pmax_idx = load_const(t_pmax_idx, [128, GPC * p.MAXN // 16],
                                  I16, "pmax_idx")

            b2col = cp.tile([128, 1], F32, tag="b2col")
            nc.vector.memset(b2col[:], ee_b2)
            epscol = cp.tile([128, 1], F32, tag="epscol")
            nc.vector.memset(epscol[:], EPS_LN)
            onescol = cp.tile([128, 1], BF, tag="onescol")
            nc.vector.memset(onescol[:], 1.0)
            ew = cp.tile([128, NDL], BF, tag="ew")
            slab = cp.tile([128, NBLK * 128], BF, tag="slab")
            dis = cp.tile([128, NBLK], F32, tag="dis")
            gmaxT = cp.tile([128, GPC], F32, tag="gmaxT")
            nc.vector.memset(gmaxT[:], 0.0)

            # ---------- DRAM scratch ----------
            tables = []  # [layer][window]
            for _k in range(L):
                tables.append([
                    dp.tile([wblk[_q] * 128 * NCORES, H], BF,
                            addr_space="Shared",
                            tag=f"table{_k}q{_q}", name=f"table{_k}q{_q}")
                    for _q in range(4)])
            slab_hbm = [dp.tile([wblk[_q] * 128, H], BF, tag=f"slabq{_q}",
                                name=f"slabq{_q}")
                        for _q in range(4)]
            h_hbm_a = dp.tile([NPC, H], F32, tag="h_hbm_a")
            h_hbm_b = dp.tile([NPC, H], F32, tag="h_hbm_b")
            h_hbm = [h_hbm_a, h_hbm_b]
            hsum_hbm = dp.tile([NPC, H], F32)
            res0_hbm = dp.tile([NPC, H], F32)
            pool_tab = dp.tile([NPC + 16, H], BF)
            opart = dp.tile([GPC, NCLS], F32)
            gout = dp.tile([NGRAPH, NCLS], F32, addr_space="Shared")

            def maybe_store_slab(b):
                if b + 1 in [int(x) for x in woff[1:]]:
                    q = [int(x) for x in woff[1:]].index(b + 1)
                    nc.sync.dma_start(
                        slab_hbm[q][:].rearrange("(b q) f -> q b f", q=128),
                        slab[:, int(woff[q]) * 128:int(woff[q + 1]) * 128]
                        .rearrange("p (b f) -> p b f", f=H))

            def emit_ag(k, qq):
                nc.gpsimd.collective_compute(
                    "AllGather", AOP.bypass,
                    replica_groups=[list(range(NCORES))],
                    ins=[slab_hbm[qq][:].opt()],
                    outs=[tables[k][qq][:].opt()])

            def emit_ags(k):
                for qq in range(4):
                    emit_ag(k, qq)

            def gather_in_ap(k, s):
                return tables[k][s][:, :]

            def build_mask(mk, b):
                dl0 = int(cell_dl[b * 4])
                cb = int(C_b[b])
                if cb == 0:
                    return None
                mk3 = mk[:, :128 * cb].rearrange("p (d c) -> p d c", c=cb)
                nc.vector.tensor_tensor(
                    out=mk3,
                    in0=iota3[:, :].rearrange("p (d c) -> p d c", c=Cmax)[:, :, :cb],
                    in1=dstloc[:, dl0:dl0 + cb].unsqueeze(1)
                        .to_broadcast([128, 128, cb]),
                    op=AOP.is_equal)
                nc.vector.tensor_tensor(
                    out=mk3, in0=mk3,
                    in1=ew[:, dl0:dl0 + cb].unsqueeze(1)
                        .to_broadcast([128, 128, cb]),
                    op=AOP.mult)
                return mk3

            # =============== preamble: edge MLP + degree + hws0 ===============
            for b in range(NBLK):
                dl0 = int(cell_dl[b * 4])
                cb = int(C_b[b])
                cols = cb * 128
                if cb == 0:
                    nc.vector.memset(dis[:, b:b + 1], 1.0)
                    bs = slice(b * 128, (b + 1) * 128)
                    xtb = bp.tile([IN, 128], F32, tag="xtb")
                    nc.sync.dma_start(xtb[:], t_xT[:, bs])
                    hw_ps = pmm.tile([128, 512], F32, tag="mm")
                    nc.tensor.matmul(hw_ps[:, :H], lhsT=xtb[:, :],
                                     rhs=W0[:, :], start=True, stop=True)
                    nc.scalar.mul(slab[:, bs], hw_ps[:, :H], dis[:, b:b + 1])
                    rs_ps = pmm.tile([128, 512], F32, tag="mm")
                    nc.tensor.matmul(rs_ps[:, :H], lhsT=xtb[:, :],
                                     rhs=resW[:, :], start=True, stop=True)
                    r0 = bp.tile([128, H], F32, tag="r0")
                    nc.vector.tensor_tensor(
                        out=r0[:], in0=rs_ps[:, :H],
                        in1=rows[:, 12 * H:13 * H], op=AOP.add)
                    nc.sync.dma_start(res0_hbm[bs, :], r0[:])
                    maybe_store_slab(b)
                    continue
                # --- edge MLP for this block's chunks ---
                ea_t = eap.tile([8, Cmax * 128], BF, tag="ea")
                nc.sync.dma_start(ea_t[:, :cols],
                                  t_eaT[:, dl0 * 128:dl0 * 128 + cols])
                eh = ehp.tile([128, 128 * Cmax], BF, tag="eh")
                for c0 in range(0, cols, 512):
                    c1 = min(c0 + 512, cols)
                    eh_ps = pmm.tile([128, 512], F32, tag="mm")
                    nc.tensor.matmul(eh_ps[:, :c1 - c0], lhsT=eeW1[:, :],
                                     rhs=ea_t[:, c0:c1], start=True, stop=True)
                    nc.scalar.activation(eh[:, c0:c1], eh_ps[:, :c1 - c0],
                                         ACTF.Relu, bias=eeb1[:, :], scale=1.0)
                ewz_ps = psm.tile([128, Cmax], F32, tag="sm")
                for c in range(cb):
                    nc.tensor.matmul(ewz_ps[:, c:c + 1],
                                     lhsT=eh[:, c * 128:(c + 1) * 128],
                                     rhs=eeW2[:, :], start=True, stop=True,
                                     skip_group_check=True)
                # softplus(z + b2) + 1e-4, into block-local ewb + persistent ew
                ezb = bp.tile([128, Cmax], F32, tag="ezb")
                nc.scalar.activation(ezb[:, :cb], ewz_ps[:, :cb], ACTF.Exp,
                                     bias=b2col[:, :], scale=1.0)
                ewb = bp.tile([128, Cmax], BF, tag="ewb")
                nc.scalar.activation(ewb[:, :cb], ezb[:, :cb], ACTF.Ln,
                                     bias=1.0, scale=1.0)
                nc.vector.tensor_scalar(ewb[:, :cb], ewb[:, :cb], 1e-4,
                                        None, AOP.add)
                nc.scalar.activation(ew[:, dl0:dl0 + cb], ewb[:, :cb],
                                     ACTF.Identity, bias=0.0, scale=1.0)

                # --- weighted mask (block-local ew) + degree ---
                mk = maskp.tile([128, 128 * Cmax], BF, tag="mask")
                mk3 = mk[:, :128 * cb].rearrange("p (d c) -> p d c", c=cb)
                nc.vector.tensor_tensor(
                    out=mk3,
                    in0=iota3[:, :].rearrange("p (d c) -> p d c", c=Cmax)[:, :, :cb],
                    in1=dstloc[:, dl0:dl0 + cb].unsqueeze(1)
                        .to_broadcast([128, 128, cb]),
                    op=AOP.is_equal)
                nc.vector.tensor_tensor(
                    out=mk3, in0=mk3,
                    in1=ewb[:, :cb].unsqueeze(1).to_broadcast([128, 128, cb]),
                    op=AOP.mult)
                deg_ps = psm.tile([128, Cmax], F32, tag="sm")
                for c in range(cb):
                    nc.tensor.matmul(
                        deg_ps[:, :1], lhsT=mk3[:, :, c],
                        rhs=onescol[:, :],
                        start=(c == 0), stop=(c == cb - 1),
                        skip_group_check=True)
                lntmp = bp.tile([128, 1], F32, tag="lntmp")
                nc.scalar.activation(lntmp[:], deg_ps[:, :1], ACTF.Ln,
                                     bias=1.0, scale=1.0)
                nc.scalar.activation(dis[:, b:b + 1], lntmp[:], ACTF.Exp,
                                     bias=0.0, scale=-0.5)

                # --- hws0 / res0 ---
                bs = slice(b * 128, (b + 1) * 128)
                xtb = bp.tile([IN, 128], F32, tag="xtb")
                nc.sync.dma_start(xtb[:], t_xT[:, bs])
                hw_ps = pmm.tile([128, 512], F32, tag="mm")
                nc.tensor.matmul(hw_ps[:, :H], lhsT=xtb[:, :],
                                 rhs=W0[:, :], start=True, stop=True)
                nc.scalar.mul(slab[:, bs], hw_ps[:, :H], dis[:, b:b + 1])
                rs_ps = pmm.tile([128, 512], F32, tag="mm")
                nc.tensor.matmul(rs_ps[:, :H], lhsT=xtb[:, :],
                                 rhs=resW[:, :], start=True, stop=True)
                r0 = bp.tile([128, H], F32, tag="r0")
                nc.vector.tensor_tensor(
                    out=r0[:], in0=rs_ps[:, :H],
                    in1=rows[:, 12 * H:13 * H], op=AOP.add)
                nc.sync.dma_start(res0_hbm[bs, :], r0[:])
                maybe_store_slab(b)

            # zero row for max-pool pads (pool_tab tail), before layer 3 uses it
            zr = bp.tile([16, H], BF, tag="zr")
            nc.vector.memset(zr[:], 0.0)
            nc.sync.dma_start(pool_tab[NPC:NPC + 16, :], zr[:])

            # =============== layers ===============
            GPCALL = max(1, MAXG // p.MAXN)
            CPG = p.MAXN // 128          # chunks per graph

            def maxpool_group(j0):
                j1 = min(j0 + GPCALL, GPC)
                nidx = (j1 - j0) * p.MAXN
                pg = msgp.tile([128, 8, 128], BF, tag="poolmax")
                nc.gpsimd.dma_gather(
                    out_ap=pg[:, :nidx // 128, :],
                    in_ap=pool_tab[:],
                    idxs_ap=pmax_idx[:, j0 * p.MAXN // 16:
                                     j0 * p.MAXN // 16 + nidx // 16],
                    num_idxs=nidx, num_idxs_reg=nidx,
                    elem_size=H, queue_num=next_q())
                for j in range(j0, j1):
                    tp = psm.tile([128, 128 * CPG], BF, tag="sm")
                    for cc in range(CPG):
                        ch = pg[:, (j - j0) * CPG + cc, :]
                        nc.tensor.transpose(tp[:, cc * 128:(cc + 1) * 128],
                                            ch, ident_bf[:, :])
                    nc.vector.reduce_max(gmaxT[:, j:j + 1], tp[:], axis=AXX)

            for k in range(L):
                gbase = np.concatenate([[0], np.cumsum(p.G_s)]).astype(int)
                issued = [0, 0, 0, 0]
                gtiles = {}

                def issue_gather(s):
                    gg = int(gbase[s]) + issued[s]
                    msg = msgp.tile([128, 8, 128], BF, tag="msg")
                    nc.gpsimd.dma_gather(
                        out_ap=msg[:, :, :],
                        in_ap=gather_in_ap(k, s),
                        idxs_ap=idx_sb[:, gg * 64:(gg + 1) * 64],
                        num_idxs=MAXG, num_idxs_reg=MAXG,
                        elem_size=H, queue_num=next_q())
                    gtiles[gg] = msg
                    issued[s] += 1

                emit_ags(k)

                for b in range(NBLK):
                    # ensure gathers covering this block are issued
                    for s in range(4 if not DBG_NOGATHER else 0):
                        while issued[s] < int(p.need_g[b, s]):
                            issue_gather(s)

                    dl0 = int(cell_dl[b * 4])
                    cb = int(C_b[b])
                    if DBG_NOGATHER or DBG_NOMASK:
                        cb = 0
                    if cb:
                        mk = maskp.tile([128, 128 * Cmax], BF, tag="mask")
                        mk3 = build_mask(mk, b)

                    agg = pag.tile([128, H], F32, tag="agg")
                    for ci in range(cb):
                        j = dl0 + ci
                        msg = gtiles[int(p.chunk_gather[j])]
                        nc.tensor.matmul(
                            agg[:], lhsT=mk3[:, :, ci],
                            rhs=msg[:, int(p.chunk_slot[j]), :],
                            start=(ci == 0), stop=False,
                            skip_group_check=True)
                    bs = slice(b * 128, (b + 1) * 128)
                    nc.tensor.matmul(agg[:], lhsT=ident_bf[:, :],
                                     rhs=slab[:, bs],
                                     start=(cb == 0), stop=True,
                                     skip_group_check=True)

                    # ---------- epilogue ----------
                    u = bp.tile([128, H], F32, tag="u")
                    nc.scalar.mul(u[:], agg[:], dis[:, b:b + 1])
                    nc.vector.tensor_tensor(
                        out=u[:], in0=u[:],
                        in1=rows[:, k * H:(k + 1) * H], op=AOP.add)
                    mu = bp.tile([128, 1], F32, tag="mu")
                    nc.vector.reduce_sum(mu[:], u[:], axis=AXX)
                    nc.vector.tensor_scalar(mu[:], mu[:], -1.0 / H, None, AOP.mult)
                    xc = bp.tile([128, H], F32, tag="xc")
                    nc.scalar.activation(xc[:], u[:], ACTF.Identity,
                                         bias=mu[:, :], scale=1.0)
                    sq = bp.tile([128, H], F32, tag="sq")
                    var = bp.tile([128, 1], F32, tag="var")
                    nc.scalar.activation(sq[:], xc[:], ACTF.Square,
                                         bias=0.0, scale=1.0, accum_out=var[:])
                    lnv = bp.tile([128, 1], F32, tag="lnv")
                    nc.scalar.activation(lnv[:], var[:], ACTF.Ln,
                                         bias=epscol[:, :], scale=1.0 / H)
                    inv = bp.tile([128, 1], F32, tag="inv")
                    nc.scalar.activation(inv[:], lnv[:], ACTF.Exp,
                                         bias=0.0, scale=-0.5)
                    y = bp.tile([128, H], F32, tag="y")
                    nc.scalar.mul(y[:], xc[:], inv[:, :])
                    nc.vector.tensor_tensor(
                        out=y[:], in0=y[:],
                        in1=rows[:, (4 + k) * H:(5 + k) * H], op=AOP.mult)
                    nc.vector.tensor_tensor(
                        out=y[:], in0=y[:],
                        in1=rows[:, (8 + k) * H:(9 + k) * H], op=AOP.add)
                    res = bp.tile([128, H], F32, tag="res")
                    if k == 0:
                        nc.sync.dma_start(res[:], res0_hbm[bs, :])
                    else:
                        nc.sync.dma_start(res[:], h_hbm[(k - 1) % 2][bs, :])
                    h = bp.tile([128, H], F32, tag="h")
                    nc.vector.tensor_tensor(out=h[:], in0=y[:], in1=res[:],
                                            op=AOP.add)
                    nc.scalar.activation(h[:], h[:], ACTF.Relu, bias=0.0, scale=1.0)
                    # hsum accumulation in HBM
                    if k == 0:
                        nc.sync.dma_start(hsum_hbm[bs, :], h[:])
                    else:
                        hs = bp.tile([128, H], F32, tag="hs")
                        nc.sync.dma_start(hs[:], hsum_hbm[bs, :])
                        nc.vector.tensor_tensor(out=hs[:], in0=hs[:], in1=h[:],
                                                op=AOP.add)
                        if k < L - 1:
                            nc.sync.dma_start(hsum_hbm[bs, :], hs[:])
                        else:
                            xm = bp.tile([128, H], BF, tag="xm")
                            nc.scalar.activation(xm[:], hs[:], ACTF.Identity,
                                                 bias=0.0, scale=0.25)
                            nc.sync.dma_start(pool_tab[bs, :], xm[:])
                            # mean pooling via selector matmul (PSUM chain)
                            if b == 0:
                                gmean_ps = pgm.tile([GPC, 512], F32, tag="gmean")
                                if DBG_NOGMEAN:
                                    nc.vector.memset(gmean_ps[:, :H], 0.0)
                            if not DBG_NOGMEAN:
                                nc.tensor.matmul(
                                    gmean_ps[:, :H],
                                    lhsT=gsel[:, b * GPC:(b + 1) * GPC],
                                    rhs=xm[:], start=(b == 0), stop=(b == NBLK - 1),
                                    skip_group_check=True)
                            # interleave ready max-pool gather groups
                            if not DBG_NOMAXPOOL:
                                for j0 in range(0, GPC, GPCALL):
                                    if int(p.pool_grp_blk[j0]) == b:
                                        maxpool_group(j0)
                    if k < L - 1:
                        nc.sync.dma_start(h_hbm[k % 2][bs, :], h[:])
                        hT_ps = pmm.tile([128, 512], F32, tag="mm")
                        nc.tensor.transpose(hT_ps[:, :H], h[:], ident_f[:, :])
                        hT = bp.tile([128, H], F32, tag="hT")
                        nc.scalar.activation(hT[:], hT_ps[:, :H], ACTF.Identity,
                                             bias=0.0, scale=1.0)
                        hw_ps = pmm.tile([128, 512], F32, tag="mm")
                        nc.tensor.matmul(hw_ps[:, :H], lhsT=hT[:],
                                         rhs=Wk[:, k * H:(k + 1) * H],
                                         start=True, stop=True)
                        nc.scalar.mul(slab[:, bs], hw_ps[:, :H], dis[:, b:b + 1])
                        maybe_store_slab(b)

            # =============== per-core head on own 64 graphs ===============
            # gmean: PSUM [GPC, H] -> SBUF, transpose to [128 h, GPC]
            gmean_sb = bp.tile([GPC, H], F32, tag="gmean_sb")
            nc.scalar.activation(gmean_sb[:], gmean_ps[:, :H], ACTF.Identity,
                                 bias=0.0, scale=1.0)
            gmT_ps = psm.tile([128, 128], F32, tag="sm")
            nc.tensor.transpose(gmT_ps[:, :GPC], gmean_sb[:],
                                ident_f[:GPC, :GPC])
            gmT = bp.tile([128, GPC], F32, tag="gmT")
            nc.scalar.activation(gmT[:], gmT_ps[:, :GPC], ACTF.Identity,
                                 bias=0.0, scale=1.0)
            # h1 [GPC, H] = gmean @ hW1[:H] + gmax @ hW1[H:]; gmaxT is
            # already [128 h, GPC] = the needed lhsT
            h1_ps = pmm.tile([128, 512], F32, tag="mm")
            nc.tensor.matmul(h1_ps[:GPC, :H], lhsT=gmT[:, :GPC],
                             rhs=hW1[:, 0:H], start=True, stop=False,
                             skip_group_check=True)
            nc.tensor.matmul(h1_ps[:GPC, :H], lhsT=gmaxT[:, :GPC],
                             rhs=hW1[:, H:2 * H], start=False, stop=True,
                             skip_group_check=True)
            h1 = bp.tile([GPC, H], F32, tag="h1")
            nc.vector.tensor_tensor(
                out=h1[:], in0=h1_ps[:GPC, :H],
                in1=rows[:GPC, 13 * H:14 * H], op=AOP.add)
            nc.vector.tensor_scalar(h1[:], h1[:], 0.0, None, AOP.max)
            h1T_ps = psm.tile([128, 128], F32, tag="sm")
            nc.tensor.transpose(h1T_ps[:, :GPC], h1[:], ident_f[:GPC, :GPC])
            h1T = bp.tile([128, GPC], F32, tag="h1T")
            nc.scalar.activation(h1T[:], h1T_ps[:, :GPC], ACTF.Identity,
                                 bias=0.0, scale=1.0)
            o_ps = pmm.tile([128, 512], F32, tag="mm")
            nc.tensor.matmul(o_ps[:GPC, :NCLS], lhsT=h1T[:, :GPC],
                             rhs=hW2[:, :], start=True, stop=True,
                             skip_group_check=True)
            o = bp.tile([GPC, NCLS], F32, tag="o")
            nc.vector.tensor_tensor(
                out=o[:], in0=o_ps[:GPC, :NCLS],
                in1=rows[:GPC, 14 * H:14 * H + NCLS], op=AOP.add)
            nc.sync.dma_start(opart[:, :], o[:])
            nc.gpsimd.collective_compute(
                "AllGather", AOP.bypass,
                replica_groups=[list(range(NCORES))],
                ins=[opart[:].opt()], outs=[gout[:].opt()])
            nc.sync.dma_start(t_out[:, :], gout[:, :])

    nc.compile()
    _fix_act_tables(nc)
    _split_waits(nc)
    return nc


def make_in_maps(p, w):
    rows = np.zeros((16, H), np.float32)  # replicated below
    for i in range(4):
        rows[i] = np.asarray(w[f'cb{i}'], np.float32)
        rows[4 + i] = np.asarray(w[f'g{i}'], np.float32)
        rows[8 + i] = np.asarray(w[f'be{i}'], np.float32)
    rows[12] = np.asarray(w['res_b'], np.float32)
    rows[13] = np.asarray(w['hb1'], np.float32)
    rows[14, :NCLS] = np.asarray(w['hb2'], np.float32)
    hW1 = np.asarray(w['hW1'], np.float32)          # [256, 128]
    hW1_pack = np.concatenate([hW1[:H, :], hW1[H:, :]], axis=1)  # [128, 256]
    Wk_pack = np.concatenate(
        [np.asarray(w[f'W{i}'], np.float32) for i in (1, 2, 3)], axis=1)
    shared = {
        "W0": np.asarray(w['W0'], np.float32),
        "resW": np.asarray(w['res_W'], np.float32),
        "Wk": Wk_pack,
        "rows": np.tile(rows.reshape(1, 16 * H), (128, 1)),
        "eeW1": np.asarray(w['ee_W1'], np.float32).astype(BF16),
        "eeW2": np.asarray(w['ee_W2'], np.float32).astype(BF16),
        "eeb1": np.asarray(w['ee_b1'], np.float32).reshape(H, 1),
        "hW1": hW1_pack,
        "hW2": np.asarray(w['hW2'], np.float32),
        "ident_bf": np.eye(128, dtype=np.float32).astype(BF16),
        "ident_f": np.eye(128, dtype=np.float32),
    }
    in_maps = []
    for r in range(NCORES):
        m = dict(shared)
        m.update({
            "xT": p.xT[r], "idx": p.idx_all[r], "dstloc": p.dstloc_all[r],
            "eaT": p.eaT_all[r], "iota3": p.iota3, "gsel": p.gsel[r],
            "pmax_idx": p.pmax_idx[r],
        })
        in_maps.append(m)
    return in_maps


def kernel(**inputs):
    from concourse.bass_utils import run_bass_kernel_spmd
    p = make_plan(inputs['x'], inputs['edge_index'], inputs['batch'],
                  inputs['edge_attr'])
    nc = build_nc(p, inputs)
    in_maps = make_in_maps(p, inputs)
    res = run_bass_kernel_spmd(nc, in_maps, core_ids=list(range(NCORES)),
                               trace=False)
    return np.asarray(res.results[0]["out"], np.float32).copy()


# revision 56
# speedup vs baseline: 1.0741x; 1.0741x over previous
"""GCN classifier kernel for Trainium2, 8 NeuronCores.

Strategy: graph-aligned node sharding (64 graphs/core), padded to NPC nodes.
Edges bucketed by (dst-block-of-128, src-quarter) cells; per-edge messages
are fetched from an AllGather-replicated bf16 node table with SWDGE
dma_gather using merged 1024-idx s-major streams, then aggregated per dst
block with one-hot matmuls whose lhsT is an ew-weighted mask:

  agg[dst_blk] = sum_chunks maskW[:, :, c]^T @ msg[chunk]   (+ identity
  self-loop from the resident slab)

maskW is laid out [128 edge, 128 dst, C] with the chunk dim innermost so
both mask-build ops (is_equal vs dstloc, mult by ew) are packed-innermost
tensor_tensor ops that hit the DVE 2x perf mode; the matmul reads lhsT with
a strided AP. The ew multiply rides the aggregation matmul for free.

PSUM drains run on the scalar (ACT) engine (DVE PSUM reads are slow).
AllGather is split in two halves (half-major table layout) to overlap the
collective with the second half of each layer. Mean pooling is a one-hot
matmul with 1/count baked into the selector; max pooling keeps the gather
path. LayerNorm/residual/ReLU epilogues run per 128-node block on ACT/DVE.
"""

import sys
import types

sys.path.insert(0, "/opt/trn_rl_repo")

import numpy as np
import ml_dtypes

BF16 = ml_dtypes.bfloat16

# Shim antenv.axon_hooks (missing in this image) so trace=True can work.
try:
    import antenv.axon_hooks  # noqa: F401
except ImportError:
    try:
        from trn_agent_boot.trn_boot import _ntff_profile_via_ctypes
        _hook = _ntff_profile_via_ctypes('/opt/axon/libaxon_pjrt.so')
    except Exception:
        _hook = None
    _mod = types.ModuleType('antenv.axon_hooks')
    _mod.get_axon_ntff_profile_hook = lambda: _hook
    sys.modules['antenv.axon_hooks'] = _mod

import concourse.bacc as bacc
import concourse.mybir as mybir
import concourse.tile as tile
import concourse.bass_utils as bass_utils

# No bucket access in this container.
bass_utils.upload_artifacts = lambda tmpdir: tmpdir

F32 = mybir.dt.float32
BF = mybir.dt.bfloat16
I16 = mybir.dt.int16
AOP = mybir.AluOpType
ACTF = mybir.ActivationFunctionType
AXX = mybir.AxisListType.X

NCORES = 8
H = 128        # hidden channels
IN = 96        # in channels
ED = 8         # edge dim
NCLS = 100     # classes
L = 4          # layers
NGRAPH = 512   # graphs
GPC = NGRAPH // NCORES
EPS_LN = 1e-5
MAXG = 1024    # dma_gather num_idxs hard limit (2048 wedges the device)

import os
DBG_NOGATHER = os.environ.get("K_NOGATHER") == "1"
DBG_NOMAXPOOL = os.environ.get("K_NOMAXPOOL") == "1"
DBG_NOGMEAN = os.environ.get("K_NOGMEAN") == "1"
DBG_NOMASK = os.environ.get("K_NOMASK") == "1"


def _split_waits(nc, max_waits=1):
    """This container's walrus rejects >1 sync wait per instruction; move
    extra waits onto preceding NOPs on the same engine."""
    n = 0
    for f in nc.m.functions:
        for bb in f.blocks:
            new_list = []
            for ins in bb.instructions:
                si = ins.sync_info
                if si and si.on_wait and len(si.on_wait) > max_waits:
                    waits = list(si.on_wait)
                    extra, keep = waits[:-max_waits], waits[-max_waits:]
                    for i, w in enumerate(extra):
                        nop = mybir.InstNoOp(name=f"{ins.name}-ws{i}", ins=[], outs=[])
                        nop.engine = ins.engine
                        nop.sync_info = mybir.SyncInfo(on_wait=[w], on_update=[])
                        new_list.append(nop)
                        n += 1
                    si.on_wait = keep
                new_list.append(ins)
            bb.instructions[:] = new_list
    return n


def _fix_act_tables(nc, set_id=6):
    """All activation funcs used here live in act table 6
    (natural_log_exp_and_others); the greedy per-func chooser ping-pongs
    between tables 0/5 costing ~27us per reload. Unify and dedupe."""
    removed = 0
    for f in nc.m.functions:
        for bb in f.blocks:
            new_list = []
            loaded = False
            for ins in bb.instructions:
                if isinstance(ins, mybir.InstLoadActFuncSet):
                    ins.act_func_set_id = set_id
                    si = ins.sync_info
                    has_sync = si and (si.on_wait or si.on_update)
                    if loaded and not has_sync:
                        removed += 1
                        continue
                    if loaded and has_sync:
                        nop = mybir.InstNoOp(name=ins.name + "-actdedup", ins=[], outs=[])
                        nop.engine = ins.engine
                        nop.sync_info = si
                        new_list.append(nop)
                        removed += 1
                        continue
                    loaded = True
                new_list.append(ins)
            bb.instructions[:] = new_list
    return removed


def _ru(x, m):
    return (x + m - 1) // m * m


def _wrap_idxs(idx):
    """[n] int -> [128, n//16] int16 SBUF wrap (i -> partition i%16, col i//16),
    replicated over the 8 gpsimd cores."""
    n = len(idx)
    assert n % 16 == 0
    a = np.asarray(idx, np.int16).reshape(n // 16, 16).T.copy()
    return np.tile(a, (8, 1))


class Plan:
    pass


def make_plan(x, edge_index, batch, edge_attr):
    N = x.shape[0]
    E = edge_index.shape[1]
    p = Plan()
    p.N, p.E = N, E

    batch = np.asarray(batch, np.int64)
    src = np.asarray(edge_index[0], np.int64)
    dst = np.asarray(edge_index[1], np.int64)

    node_start = np.searchsorted(batch, np.arange(NGRAPH + 1))  # [513]
    core_start = node_start[::GPC][:NCORES].astype(np.int64)
    core_end = np.append(core_start[1:], N).astype(np.int64)
    core_cnt = core_end - core_start
    NPC = max(512, _ru(int(core_cnt.max()), 128))
    NBLK = NPC // 128
    NPAD = NCORES * NPC
    # Four gather windows (int16 idx => window rows <= 32767), uneven: the
    # last window is small so its AllGather (the only exposed one at layer
    # boundaries) is short.
    MAXWB = 32767 // (NCORES * 128)  # max window size in blocks (31)
    base = max(1, min(MAXWB, round(NBLK * 0.26)))
    wblk = [base, base, base, NBLK - 3 * base]
    assert 1 <= wblk[3] <= MAXWB, (NBLK, wblk)
    woff = np.concatenate([[0], np.cumsum(wblk)]).astype(np.int64)
    p.wblk, p.woff = wblk, woff
    p.NPC, p.NBLK, p.NPAD = NPC, NBLK, NPAD
    p.core_start, p.core_cnt = core_start, core_cnt

    owner = np.searchsorted(core_start, np.arange(N), side='right') - 1
    loc = np.arange(N) - core_start[owner]
    # window-major table row: table window w holds every core's local node
    # blocks [woff[w], woff[w+1]), and equals gather window w.
    blk_of = loc >> 7
    wsel = np.searchsorted(woff, blk_of, side='right') - 1
    table_row = (woff[wsel] * NCORES * 128 + owner * (woff[wsel + 1] - woff[wsel]) * 128
                 + (loc - woff[wsel] * 128))

    src_t_all = table_row[src]
    srange_all = wsel[src]
    d_owner = owner[dst]
    d_loc = loc[dst]
    blk_all = d_loc >> 7
    dloc_all = d_loc & 127
    cell_all = blk_all * 4 + srange_all  # b-major cell id
    NCELL = NBLK * 4

    order = np.lexsort((src_t_all, cell_all, d_owner))
    src_t = src_t_all[order]
    dloc = dloc_all[order]
    cell = cell_all[order]
    e_owner = d_owner[order]
    ea_perm = np.asarray(edge_attr, np.float32)[order]

    counts = np.zeros((NCORES, NCELL), np.int64)
    for r in range(NCORES):
        m = e_owner == r
        counts[r] = np.bincount(cell[m], minlength=NCELL)
    core_off = np.searchsorted(e_owner, np.arange(NCORES + 1))
    core_cell_off = np.zeros((NCORES, NCELL + 1), np.int64)
    for r in range(NCORES):
        core_cell_off[r, 0] = core_off[r]
        core_cell_off[r, 1:] = np.cumsum(counts[r]) + core_off[r]

    cnum = _ru(counts.max(axis=0), 128)     # padded idx count per cell
    Cg = cnum // 128                        # chunks per cell
    p.Cg = Cg

    # b-major chunk columns
    cell_dl = np.concatenate([[0], np.cumsum(Cg)]).astype(np.int64)
    NDL = int(cell_dl[-1])
    p.NDL = NDL
    C_b = np.array([int(cell_dl[b * 4 + 4] - cell_dl[b * 4]) for b in range(NBLK)])
    p.C_b = C_b
    p.Cmax = int(C_b.max())
    assert p.Cmax >= 2

    # s-major gather streams: stream s = concat over b of cell (b, s) chunks
    stream_cells = [[b * 4 + s for b in range(NBLK)] for s in range(4)]
    stream_len = [int(sum(Cg[c] for c in cs)) for cs in stream_cells]  # chunks
    G_s = [(sl * 128 + MAXG - 1) // MAXG for sl in stream_len]
    p.G_s = G_s
    gbase = np.concatenate([[0], np.cumsum(G_s)]).astype(np.int64)
    p.NGATH = int(gbase[-1])

    # chunk (b-major col j) -> (gather g, slot) and per-(b,s) gather needs
    chunk_gather = np.zeros(NDL, np.int64)
    chunk_slot = np.zeros(NDL, np.int64)
    need_g = np.zeros((NBLK, 4), np.int64)   # gathers of stream s needed
    pos_s = [0, 0, 0, 0]
    for b in range(NBLK):
        for s in range(4):
            c = b * 4 + s
            for k in range(Cg[c]):
                j = cell_dl[c] + k
                pos = pos_s[s]
                chunk_gather[j] = gbase[s] + pos // 8
                chunk_slot[j] = pos % 8
                pos_s[s] += 1
            need_g[b, s] = (pos_s[s] + 7) // 8  # ceil chunks/8 so far
    p.chunk_gather, p.chunk_slot, p.need_g = chunk_gather, chunk_slot, need_g

    # fill per-core data
    dstloc_f = np.full((NCORES, 128, NDL), 255.0, np.float32)
    p.eaT_all = np.zeros((NCORES, 8, NDL * 128), BF16)
    idx_stream = np.zeros((NCORES, 4, max(G_s) * MAXG), np.int64)
    for r in range(NCORES):
        spos = [0, 0, 0, 0]
        for b in range(NBLK):
            for s in range(4):
                c = b * 4 + s
                if Cg[c] == 0:
                    continue
                a0 = core_cell_off[r, c]
                a1 = core_cell_off[r, c + 1]
                n_real = int(a1 - a0)
                num = int(cnum[c])
                # idxs for this cell (pad slots -> 0)
                iv = np.zeros(num, np.int64)
                if n_real:
                    iv[:n_real] = src_t[a0:a1] - int(woff[s]) * NCORES * 128
                idx_stream[r, s, spos[s]:spos[s] + num] = iv
                spos[s] += num
                # dstloc cols (b-major)
                dl = np.full(num, 255.0, np.float32)
                if n_real:
                    dl[:n_real] = dloc[a0:a1].astype(np.float32)
                dstloc_f[r, :, cell_dl[c]:cell_dl[c + 1]] = \
                    dl.reshape(Cg[c], 128).T
                # edge attrs (b-major)
                if n_real:
                    ea = np.zeros((num, ED), np.float32)
                    ea[:n_real] = ea_perm[a0:a1]
                    p.eaT_all[r, :, cell_dl[c] * 128:cell_dl[c + 1] * 128] = \
                        ea.T.astype(BF16)
    p.dstloc_all = dstloc_f.astype(BF16)

    # wrap idx streams into gather-major int16 [128, NGATH*64]
    p.idx_all = np.zeros((NCORES, 128, p.NGATH * (MAXG // 16)), np.int16)
    for r in range(NCORES):
        for s in range(4):
            for g in range(G_s[s]):
                iv = idx_stream[r, s, g * MAXG:(g + 1) * MAXG]
                gg = int(gbase[s]) + g
                p.idx_all[r, :, gg * 64:(gg + 1) * 64] = _wrap_idxs(iv)

    # x slab, transposed [96, NPC] per core
    p.xT = np.zeros((NCORES, IN, NPC), np.float32)
    xf = np.asarray(x, np.float32)
    for r in range(NCORES):
        p.xT[r, :, :core_cnt[r]] = xf[core_start[r]:core_end[r]].T

    # mean pooling selector: gsel[node p of block b, g] = 1/count(g)
    gcnt = (node_start[1:] - node_start[:-1]).astype(np.int64)
    p.gsel = np.zeros((NCORES, 128, NBLK * GPC), np.float32)
    for r in range(NCORES):
        for lid in range(int(core_cnt[r])):
            g = int(batch[core_start[r] + lid])
            jl = g - r * GPC
            b, pp = lid >> 7, lid & 127
            p.gsel[r, pp, b * GPC + jl] = 1.0 / max(int(gcnt[g]), 1)
    p.gsel = p.gsel.astype(BF16)

    # max pooling: gather idx per graph padded to MAXN (repeat first node)
    MAXN = max(128, _ru(int(gcnt.max()), 128))
    assert MAXN <= MAXG
    p.MAXN = MAXN
    ZROW = NPC
    p.pmax_idx = np.zeros((NCORES, 128, GPC * MAXN // 16), np.int16)
    for r in range(NCORES):
        mi = []
        for j in range(GPC):
            gid = r * GPC + j
            a = int(node_start[gid] - core_start[r])
            n = int(gcnt[gid])
            ids = np.arange(a, a + n)
            pad = MAXN - n
            mi.append(np.concatenate([ids, np.full(pad, ids[0] if n else ZROW)]))
        p.pmax_idx[r] = _wrap_idxs(np.concatenate(mi))
    # block (uniform across cores) after which max-gather group j0 can fire
    GPCALL = max(1, MAXG // MAXN)
    p.pool_grp_blk = np.zeros(GPC, np.int64)
    for j0 in range(0, GPC, GPCALL):
        j1 = min(j0 + GPCALL, GPC)
        endmax = 0
        for r in range(NCORES):
            e = int(node_start[min(r * GPC + j1, NGRAPH)] - core_start[r])
            endmax = max(endmax, (e + 127) // 128)
        p.pool_grp_blk[j0] = min(endmax, NBLK) - 1

    # iota3 [128, 128, Cmax] bf16: value d at (p, d, c)
    p.iota3 = np.tile(
        np.arange(128, dtype=np.float32)[None, :, None],
        (128, 1, p.Cmax)).reshape(128, 128 * p.Cmax).astype(BF16)
    return p


def build_nc(p, w):
    nc = bacc.Bacc("TRN2", num_devices=NCORES, detect_race_conditions=False,
                   num_swdge_queues=4)
    NPC, NBLK, NPAD = p.NPC, p.NBLK, p.NPAD
    wblk, woff = p.wblk, p.woff
    NDL, Cmax, C_b, Cg = p.NDL, p.Cmax, p.C_b, p.Cg
    cell_dl = np.concatenate([[0], np.cumsum(Cg)]).astype(np.int64)

    # ---- I/O ----
    t_xT = nc.dram_tensor("xT", [IN, NPC], F32, kind="ExternalInput")
    t_idx = nc.dram_tensor("idx", [128, p.NGATH * 64], I16, kind="ExternalInput")
    t_dstloc = nc.dram_tensor("dstloc", [128, NDL], BF, kind="ExternalInput")
    t_eaT = nc.dram_tensor("eaT", [8, NDL * 128], BF, kind="ExternalInput")
    t_iota3 = nc.dram_tensor("iota3", [128, 128 * Cmax], BF, kind="ExternalInput")
    t_gsel = nc.dram_tensor("gsel", [128, NBLK * GPC], BF, kind="ExternalInput")
    t_pmax_idx = nc.dram_tensor("pmax_idx", [128, GPC * p.MAXN // 16], I16,
                                kind="ExternalInput")
    t_W0 = nc.dram_tensor("W0", [IN, H], F32, kind="ExternalInput")
    t_resW = nc.dram_tensor("resW", [IN, H], F32, kind="ExternalInput")
    t_Wk = nc.dram_tensor("Wk", [H, 3 * H], F32, kind="ExternalInput")
    t_rows = nc.dram_tensor("rows", [128, 16 * H], F32, kind="ExternalInput")
    t_eeW1 = nc.dram_tensor("eeW1", [ED, H], BF, kind="ExternalInput")
    t_eeW2 = nc.dram_tensor("eeW2", [H, 1], BF, kind="ExternalInput")
    t_eeb1 = nc.dram_tensor("eeb1", [H, 1], F32, kind="ExternalInput")
    t_hW1 = nc.dram_tensor("hW1", [H, 2 * H], F32, kind="ExternalInput")
    t_hW2 = nc.dram_tensor("hW2", [H, NCLS], F32, kind="ExternalInput")
    t_ident_bf = nc.dram_tensor("ident_bf", [128, 128], BF, kind="ExternalInput")
    t_ident_f = nc.dram_tensor("ident_f", [128, 128], F32, kind="ExternalInput")
    t_out = nc.dram_tensor("out", [NGRAPH, NCLS], F32, kind="ExternalOutput")

    ee_b2 = float(np.asarray(w['ee_b2']).reshape(-1)[0])

    gq_counter = [0]

    def next_q():
        q = gq_counter[0] % 4
        gq_counter[0] += 1
        return q

    with tile.TileContext(nc) as tc:
        with (
            tc.tile_pool(name="const", bufs=1) as cp,
            tc.tile_pool(name="dram", bufs=1, space="DRAM") as dp,
            tc.tile_pool(name="ea", bufs=2) as eap,
            tc.tile_pool(name="eh", bufs=2) as ehp,
            tc.tile_pool(name="msg", bufs=13) as msgp,
            tc.tile_pool(name="mask", bufs=3) as maskp,
            tc.tile_pool(name="blk", bufs=3) as bp,
            tc.tile_pool(name="pag", bufs=3, space="PSUM") as pag,
            tc.tile_pool(name="pgm", bufs=1, space="PSUM") as pgm,
            tc.tile_pool(name="pmm", bufs=2, space="PSUM") as pmm,
            tc.tile_pool(name="psm", bufs=2, space="PSUM") as psm,
        ):
            # ---------- resident tiles ----------
            def load_const(t, shape, dtype, tag):
                tl = cp.tile(shape, dtype, tag=tag)
                nc.sync.dma_start(tl[:], t[:])
                return tl

            idx_sb = load_const(t_idx, [128, p.NGATH * 64], I16, "idx_sb")
            dstloc = load_const(t_dstloc, [128, NDL], BF, "dstloc")
            iota3 = load_const(t_iota3, [128, 128 * Cmax], BF, "iota3")
            gsel = load_const(t_gsel, [128, NBLK * GPC], BF, "gsel")
            W0 = load_const(t_W0, [IN, H], F32, "W0")
            resW = load_const(t_resW, [IN, H], F32, "resW")
            Wk = load_const(t_Wk, [H, 3 * H], F32, "Wk")
            rows = load_const(t_rows, [128, 16 * H], F32, "rows")
            eeW1 = load_const(t_eeW1, [ED, H], BF, "eeW1")
            eeW2 = load_const(t_eeW2, [H, 1], BF, "eeW2")
            eeb1 = load_const(t_eeb1, [H, 1], F32, "eeb1")
            hW1 = load_const(t_hW1, [H, 2 * H], F32, "hW1")
            hW2 = load_const(t_hW2, [H, NCLS], F32, "hW2")
            ident_bf = load_const(t_ident_bf, [128, 128], BF, "ident_bf")
            ident_f = load_const(t_ident_f, [128, 128], F32, "ident_f")
            pmax_idx = load_const(t_pmax_idx, [128, GPC * p.MAXN // 16],
                                  I16, "pmax_idx")

            b2col = cp.tile([128, 1], F32, tag="b2col")
            nc.vector.memset(b2col[:], ee_b2)
            epscol = cp.tile([128, 1], F32, tag="epscol")
            nc.vector.memset(epscol[:], EPS_LN)
            onescol = cp.tile([128, 1], BF, tag="onescol")
            nc.vector.memset(onescol[:], 1.0)
            ew = cp.tile([128, NDL], BF, tag="ew")
            slab = cp.tile([128, NBLK * 128], BF, tag="slab")
            dis = cp.tile([128, NBLK], F32, tag="dis")
            gmaxT = cp.tile([128, GPC], F32, tag="gmaxT")
            nc.vector.memset(gmaxT[:], 0.0)

            # ---------- DRAM scratch ----------
            tables = []  # [layer][window]
            for _k in range(L):
                tables.append([
                    dp.tile([wblk[_q] * 128 * NCORES, H], BF,
                            addr_space="Shared",
                            tag=f"table{_k}q{_q}", name=f"table{_k}q{_q}")
                    for _q in range(4)])
            slab_hbm = [dp.tile([wblk[_q] * 128, H], BF, tag=f"slabq{_q}",
                                name=f"slabq{_q}")
                        for _q in range(4)]
            h_hbm_a = dp.tile([NPC, H], F32, tag="h_hbm_a")
            h_hbm_b = dp.tile([NPC, H], F32, tag="h_hbm_b")
            h_hbm = [h_hbm_a, h_hbm_b]
            hsum_hbm = dp.tile([NPC, H], F32)
            res0_hbm = dp.tile([NPC, H], F32)
            pool_tab = dp.tile([NPC + 16, H], BF)
            opart = dp.tile([GPC, NCLS], F32)
            gout = dp.tile([NGRAPH, NCLS], F32, addr_space="Shared")

            def maybe_store_slab(b):
                if b + 1 in [int(x) for x in woff[1:]]:
                    q = [int(x) for x in woff[1:]].index(b + 1)
                    nc.sync.dma_start(
                        slab_hbm[q][:].rearrange("(b q) f -> q b f", q=128),
                        slab[:, int(woff[q]) * 128:int(woff[q + 1]) * 128]
                        .rearrange("p (b f) -> p b f", f=H))

            def emit_ag(k, qq):
                nc.gpsimd.collective_compute(
                    "AllGather", AOP.bypass,
                    replica_groups=[list(range(NCORES))],
                    ins=[slab_hbm[qq][:].opt()],
                    outs=[tables[k][qq][:].opt()])

            def emit_ags(k):
                for qq in range(4):
                    emit_ag(k, qq)

            def gather_in_ap(k, s):
                return tables[k][s][:, :]

            def build_mask(mk, b):
                dl0 = int(cell_dl[b * 4])
                cb = int(C_b[b])
                if cb == 0:
                    return None
                mk3 = mk[:, :128 * cb].rearrange("p (d c) -> p d c", c=cb)
                nc.vector.tensor_tensor(
                    out=mk3,
                    in0=iota3[:, :].rearrange("p (d c) -> p d c", c=Cmax)[:, :, :cb],
                    in1=dstloc[:, dl0:dl0 + cb].unsqueeze(1)
                        .to_broadcast([128, 128, cb]),
                    op=AOP.is_equal)
                nc.vector.tensor_tensor(
                    out=mk3, in0=mk3,
                    in1=ew[:, dl0:dl0 + cb].unsqueeze(1)
                        .to_broadcast([128, 128, cb]),
                    op=AOP.mult)
                return mk3

            # =============== preamble: edge MLP + degree + hws0 ===============
            for b in range(NBLK):
                dl0 = int(cell_dl[b * 4])
                cb = int(C_b[b])
                cols = cb * 128
                if cb == 0:
                    nc.vector.memset(dis[:, b:b + 1], 1.0)
                    bs = slice(b * 128, (b + 1) * 128)
                    xtb = bp.tile([IN, 128], F32, tag="xtb")
                    nc.sync.dma_start(xtb[:], t_xT[:, bs])
                    hw_ps = pmm.tile([128, 512], F32, tag="mm")
                    nc.tensor.matmul(hw_ps[:, :H], lhsT=xtb[:, :],
                                     rhs=W0[:, :], start=True, stop=True)
                    nc.scalar.mul(slab[:, bs], hw_ps[:, :H], dis[:, b:b + 1])
                    rs_ps = pmm.tile([128, 512], F32, tag="mm")
                    nc.tensor.matmul(rs_ps[:, :H], lhsT=xtb[:, :],
                                     rhs=resW[:, :], start=True, stop=True)
                    r0 = bp.tile([128, H], F32, tag="r0")
                    nc.vector.tensor_tensor(
                        out=r0[:], in0=rs_ps[:, :H],
                        in1=rows[:, 12 * H:13 * H], op=AOP.add)
                    nc.sync.dma_start(res0_hbm[bs, :], r0[:])
                    maybe_store_slab(b)
                    continue
                # --- edge MLP for this block's chunks ---
                ea_t = eap.tile([8, Cmax * 128], BF, tag="ea")
                nc.sync.dma_start(ea_t[:, :cols],
                                  t_eaT[:, dl0 * 128:dl0 * 128 + cols])
                eh = ehp.tile([128, 128 * Cmax], BF, tag="eh")
                for c0 in range(0, cols, 512):
                    c1 = min(c0 + 512, cols)
                    eh_ps = pmm.tile([128, 512], F32, tag="mm")
                    nc.tensor.matmul(eh_ps[:, :c1 - c0], lhsT=eeW1[:, :],
                                     rhs=ea_t[:, c0:c1], start=True, stop=True)
                    nc.scalar.activation(eh[:, c0:c1], eh_ps[:, :c1 - c0],
                                         ACTF.Relu, bias=eeb1[:, :], scale=1.0)
                ewz_ps = psm.tile([128, Cmax], F32, tag="sm")
                for c in range(cb):
                    nc.tensor.matmul(ewz_ps[:, c:c + 1],
                                     lhsT=eh[:, c * 128:(c + 1) * 128],
                                     rhs=eeW2[:, :], start=True, stop=True,
                                     skip_group_check=True)
                # softplus(z + b2) + 1e-4, into block-local ewb + persistent ew
                ezb = bp.tile([128, Cmax], F32, tag="ezb")
                nc.scalar.activation(ezb[:, :cb], ewz_ps[:, :cb], ACTF.Exp,
                                     bias=b2col[:, :], scale=1.0)
                ewb = bp.tile([128, Cmax], BF, tag="ewb")
                nc.scalar.activation(ewb[:, :cb], ezb[:, :cb], ACTF.Ln,
                                     bias=1.0, scale=1.0)
                nc.vector.tensor_scalar(ewb[:, :cb], ewb[:, :cb], 1e-4,
                                        None, AOP.add)
                nc.scalar.activation(ew[:, dl0:dl0 + cb], ewb[:, :cb],
                                     ACTF.Identity, bias=0.0, scale=1.0)

                # --- weighted mask (block-local ew) + degree ---
                mk = maskp.tile([128, 128 * Cmax], BF, tag="mask")
                mk3 = mk[:, :128 * cb].rearrange("p (d c) -> p d c", c=cb)
                nc.vector.tensor_tensor(
                    out=mk3,
                    in0=iota3[:, :].rearrange("p (d c) -> p d c", c=Cmax)[:, :, :cb],
                    in1=dstloc[:, dl0:dl0 + cb].unsqueeze(1)
                        .to_broadcast([128, 128, cb]),
                    op=AOP.is_equal)
                nc.vector.tensor_tensor(
                    out=mk3, in0=mk3,
                    in1=ewb[:, :cb].unsqueeze(1).to_broadcast([128, 128, cb]),
                    op=AOP.mult)
                deg_ps = psm.tile([128, Cmax], F32, tag="sm")
                for c in range(cb):
                    nc.tensor.matmul(
                        deg_ps[:, :1], lhsT=mk3[:, :, c],
                        rhs=onescol[:, :],
                        start=(c == 0), stop=(c == cb - 1),
                        skip_group_check=True)
                lntmp = bp.tile([128, 1], F32, tag="lntmp")
                nc.scalar.activation(lntmp[:], deg_ps[:, :1], ACTF.Ln,
                                     bias=1.0, scale=1.0)
                nc.scalar.activation(dis[:, b:b + 1], lntmp[:], ACTF.Exp,
                                     bias=0.0, scale=-0.5)

                # --- hws0 / res0 ---
                bs = slice(b * 128, (b + 1) * 128)
                xtb = bp.tile([IN, 128], F32, tag="xtb")
                nc.sync.dma_start(xtb[:], t_xT[:, bs])
                hw_ps = pmm.tile([128, 512], F32, tag="mm")
                nc.tensor.matmul(hw_ps[:, :H], lhsT=xtb[:, :],
                                 rhs=W0[:, :], start=True, stop=True)
                nc.scalar.mul(slab[:, bs], hw_ps[:, :H], dis[:, b:b + 1])
                rs_ps = pmm.tile([128, 512], F32, tag="mm")
                nc.tensor.matmul(rs_ps[:, :H], lhsT=xtb[:, :],
                                 rhs=resW[:, :], start=True, stop=True)
                r0 = bp.tile([128, H], F32, tag="r0")
                nc.vector.tensor_tensor(
                    out=r0[:], in0=rs_ps[:, :H],
                    in1=rows[:, 12 * H:13 * H], op=AOP.add)
                nc.sync.dma_start(res0_hbm[bs, :], r0[:])
                maybe_store_slab(b)

            # zero row for max-pool pads (pool_tab tail), before layer 3 uses it
            zr = bp.tile([16, H], BF, tag="zr")
            nc.vector.memset(zr[:], 0.0)
            nc.sync.dma_start(pool_tab[NPC:NPC + 16, :], zr[:])

            # =============== layers ===============
            GPCALL = max(1, MAXG // p.MAXN)
            CPG = p.MAXN // 128          # chunks per graph

            def maxpool_group(j0):
                j1 = min(j0 + GPCALL, GPC)
                nidx = (j1 - j0) * p.MAXN
                pg = msgp.tile([128, 8, 128], BF, tag="poolmax")
                nc.gpsimd.dma_gather(
                    out_ap=pg[:, :nidx // 128, :],
                    in_ap=pool_tab[:],
                    idxs_ap=pmax_idx[:, j0 * p.MAXN // 16:
                                     j0 * p.MAXN // 16 + nidx // 16],
                    num_idxs=nidx, num_idxs_reg=nidx,
                    elem_size=H, queue_num=next_q())
                for j in range(j0, j1):
                    tp = psm.tile([128, 128 * CPG], BF, tag="sm")
                    for cc in range(CPG):
                        ch = pg[:, (j - j0) * CPG + cc, :]
                        nc.tensor.transpose(tp[:, cc * 128:(cc + 1) * 128],
                                            ch, ident_bf[:, :])
                    nc.vector.reduce_max(gmaxT[:, j:j + 1], tp[:], axis=AXX)

            for k in range(L):
                gbase = np.concatenate([[0], np.cumsum(p.G_s)]).astype(int)
                issued = [0, 0, 0, 0]
                gtiles = {}

                def issue_gather(s):
                    gg = int(gbase[s]) + issued[s]
                    msg = msgp.tile([128, 8, 128], BF, tag="msg")
                    nc.gpsimd.dma_gather(
                        out_ap=msg[:, :, :],
                        in_ap=gather_in_ap(k, s),
                        idxs_ap=idx_sb[:, gg * 64:(gg + 1) * 64],
                        num_idxs=MAXG, num_idxs_reg=MAXG,
                        elem_size=H, queue_num=next_q())
                    gtiles[gg] = msg
                    issued[s] += 1

                emit_ags(k)

                for b in range(NBLK):
                    # ensure gathers covering this block are issued
                    for s in range(4 if not DBG_NOGATHER else 0):
                        while issued[s] < int(p.need_g[b, s]):
                            issue_gather(s)

                    dl0 = int(cell_dl[b * 4])
                    cb = int(C_b[b])
                    if DBG_NOGATHER or DBG_NOMASK:
                        cb = 0
                    if cb:
                        mk = maskp.tile([128, 128 * Cmax], BF, tag="mask")
                        mk3 = build_mask(mk, b)

                    agg = pag.tile([128, H], F32, tag="agg")
                    for ci in range(cb):
                        j = dl0 + ci
                        msg = gtiles[int(p.chunk_gather[j])]
                        nc.tensor.matmul(
                            agg[:], lhsT=mk3[:, :, ci],
                            rhs=msg[:, int(p.chunk_slot[j]), :],
                            start=(ci == 0), stop=False,
                            skip_group_check=True)
                    bs = slice(b * 128, (b + 1) * 128)
                    nc.tensor.matmul(agg[:], lhsT=ident_bf[:, :],
                                     rhs=slab[:, bs],
                                     start=(cb == 0), stop=True,
                                     skip_group_check=True)

                    # ---------- epilogue ----------
                    u = bp.tile([128, H], F32, tag="u")
                    nc.scalar.mul(u[:], agg[:], dis[:, b:b + 1])
                    nc.vector.tensor_tensor(
                        out=u[:], in0=u[:],
                        in1=rows[:, k * H:(k + 1) * H], op=AOP.add)
                    mu = bp.tile([128, 1], F32, tag="mu")
                    nc.vector.reduce_sum(mu[:], u[:], axis=AXX)
                    nc.vector.tensor_scalar(mu[:], mu[:], -1.0 / H, None, AOP.mult)
                    xc = bp.tile([128, H], F32, tag="xc")
                    nc.scalar.activation(xc[:], u[:], ACTF.Identity,
                                         bias=mu[:, :], scale=1.0)
                    sq = bp.tile([128, H], F32, tag="sq")
                    var = bp.tile([128, 1], F32, tag="var")
                    nc.scalar.activation(sq[:], xc[:], ACTF.Square,
                                         bias=0.0, scale=1.0, accum_out=var[:])
                    lnv = bp.tile([128, 1], F32, tag="lnv")
                    nc.scalar.activation(lnv[:], var[:], ACTF.Ln,
                                         bias=epscol[:, :], scale=1.0 / H)
                    inv = bp.tile([128, 1], F32, tag="inv")
                    nc.scalar.activation(inv[:], lnv[:], ACTF.Exp,
                                         bias=0.0, scale=-0.5)
                    y = bp.tile([128, H], F32, tag="y")
                    nc.scalar.mul(y[:], xc[:], inv[:, :])
                    nc.vector.tensor_tensor(
                        out=y[:], in0=y[:],
                        in1=rows[:, (4 + k) * H:(5 + k) * H], op=AOP.mult)
                    nc.vector.tensor_tensor(
                        out=y[:], in0=y[:],
                        in1=rows[:, (8 + k) * H:(9 + k) * H], op=AOP.add)
                    res = bp.tile([128, H], F32, tag="res")
                    if k == 0:
                        nc.sync.dma_start(res[:], res0_hbm[bs, :])
                    else:
                        nc.sync.dma_start(res[:], h_hbm[(k - 1) % 2][bs, :])
                    h = bp.tile([128, H], F32, tag="h")
                    nc.vector.tensor_tensor(out=h[:], in0=y[:], in1=res[:],
                                            op=AOP.add)
                    nc.scalar.activation(h[:], h[:], ACTF.Relu, bias=0.0, scale=1.0)
                    # hsum accumulation in HBM
                    if k == 0:
                        nc.sync.dma_start(hsum_hbm[bs, :], h[:])
                    else:
                        hs = bp.tile([128, H], F32, tag="hs")
                        nc.sync.dma_start(hs[:], hsum_hbm[bs, :])
                        nc.vector.tensor_tensor(out=hs[:], in0=hs[:], in1=h[:],
                                                op=AOP.add)
                        if k < L - 1:
                            nc.sync.dma_start(hsum_hbm[bs, :], hs[:])
                        else:
                            xm = bp.tile([128, H], BF, tag="xm")
                            nc.scalar.activation(xm[:], hs[:], ACTF.Identity,
                                                 bias=0.0, scale=0.25)
                            nc.sync.dma_start(pool_tab[bs, :], xm[:])
                            # mean pooling via selector matmul (PSUM chain)
                            if b == 0:
                                gmean_ps = pgm.tile([GPC, 512], F32, tag="gmean")
                                if DBG_NOGMEAN:
                                    nc.vector.memset(gmean_ps[:, :H], 0.0)
                            if not DBG_NOGMEAN:
                                nc.tensor.matmul(
                                    gmean_ps[:, :H],
                                    lhsT=gsel[:, b * GPC:(b + 1) * GPC],
                                    rhs=xm[:], start=(b == 0), stop=(b == NBLK - 1),
                                    skip_group_check=True)
                            # interleave ready max-pool gather groups
                            if not DBG_NOMAXPOOL:
                                for j0 in range(0, GPC, GPCALL):
                                    if int(p.pool_grp_blk[j0]) == b:
                                        maxpool_group(j0)
                    if k < L - 1:
                        nc.sync.dma_start(h_hbm[k % 2][bs, :], h[:])
                        hT_ps = pmm.tile([128, 512], F32, tag="mm")
                        nc.tensor.transpose(hT_ps[:, :H], h[:], ident_f[:, :])
                        hT = bp.tile([128, H], F32, tag="hT")
                        nc.scalar.activation(hT[:], hT_ps[:, :H], ACTF.Identity,
                                             bias=0.0, scale=1.0)
                        hw_ps = pmm.tile([128, 512], F32, tag="mm")
                        nc.tensor.matmul(hw_ps[:, :H], lhsT=hT[:],
                                         rhs=Wk[:, k * H:(k + 1) * H],
                                         start=True, stop=True)
                        nc.scalar.mul(slab[:, bs], hw_ps[:, :H], dis[:, b:b + 1])
                        maybe_store_slab(b)

            # =============== per-core head on own 64 graphs ===============
            # gmean: PSUM [GPC, H] -> SBUF, transpose to [128 h, GPC]
            gmean_sb = bp.tile([GPC, H], F32, tag="gmean_sb")
            nc.scalar.activation(gmean_sb[:], gmean_ps[:, :H], ACTF.Identity,
                                 bias=0.0, scale=1.0)
            gmT_ps = psm.tile([128, 128], F32, tag="sm")
            nc.tensor.transpose(gmT_ps[:, :GPC], gmean_sb[:],
                                ident_f[:GPC, :GPC])
            gmT = bp.tile([128, GPC], F32, tag="gmT")
            nc.scalar.activation(gmT[:], gmT_ps[:, :GPC], ACTF.Identity,
                                 bias=0.0, scale=1.0)
            # h1 [GPC, H] = gmean @ hW1[:H] + gmax @ hW1[H:]; gmaxT is
            # already [128 h, GPC] = the needed lhsT
            h1_ps = pmm.tile([128, 512], F32, tag="mm")
            nc.tensor.matmul(h1_ps[:GPC, :H], lhsT=gmT[:, :GPC],
                             rhs=hW1[:, 0:H], start=True, stop=False,
                             skip_group_check=True)
            nc.tensor.matmul(h1_ps[:GPC, :H], lhsT=gmaxT[:, :GPC],
                             rhs=hW1[:, H:2 * H], start=False, stop=True,
                             skip_group_check=True)
            h1 = bp.tile([GPC, H], F32, tag="h1")
            nc.vector.tensor_tensor(
                out=h1[:], in0=h1_ps[:GPC, :H],
                in1=rows[:GPC, 13 * H:14 * H], op=AOP.add)
            nc.vector.tensor_scalar(h1[:], h1[:], 0.0, None, AOP.max)
            h1T_ps = psm.tile([128, 128], F32, tag="sm")
            nc.tensor.transpose(h1T_ps[:, :GPC], h1[:], ident_f[:GPC, :GPC])
            h1T = bp.tile([128, GPC], F32, tag="h1T")
            nc.scalar.activation(h1T[:], h1T_ps[:, :GPC], ACTF.Identity,
                                 bias=0.0, scale=1.0)
            o_ps = pmm.tile([128, 512], F32, tag="mm")
            nc.tensor.matmul(o_ps[:GPC, :NCLS], lhsT=h1T[:, :GPC],
                             rhs=hW2[:, :], start=True, stop=True,
                             skip_group_check=True)
            o = bp.tile([GPC, NCLS], F32, tag="o")
            nc.vector.tensor_tensor(
                out=o[:], in0=o_ps[:GPC, :NCLS],
                in1=rows[:GPC, 14 * H:14 * H + NCLS], op=AOP.add)
            nc.sync.dma_start(opart[:, :], o[:])
            nc.gpsimd.collective_compute(
                "AllGather", AOP.bypass,
                replica_groups=[list(range(NCORES))],
                ins=[opart[:].opt()], outs=[gout[:].opt()])
            nc.sync.dma_start(t_out[:, :], gout[:, :])

    nc.compile()
    _fix_act_tables(nc)
    _split_waits(nc)
    return nc


def make_in_maps(p, w):
    rows = np.zeros((16, H), np.float32)  # replicated below
    for i in range(4):
        rows[i] = np.asarray(w[f'cb{i}'], np.float32)
        rows[4 + i] = np.asarray(w[f'g{i}'], np.float32)
        rows[8 + i] = np.asarray(w[f'be{i}'], np.float32)
    rows[12] = np.asarray(w['res_b'], np.float32)
    rows[13] = np.asarray(w['hb1'], np.float32)
    rows[14, :NCLS] = np.asarray(w['hb2'], np.float32)
    hW1 = np.asarray(w['hW1'], np.float32)          # [256, 128]
    hW1_pack = np.concatenate([hW1[:H, :], hW1[H:, :]], axis=1)  # [128, 256]
    Wk_pack = np.concatenate(
        [np.asarray(w[f'W{i}'], np.float32) for i in (1, 2, 3)], axis=1)
    shared = {
        "W0": np.asarray(w['W0'], np.float32),
        "resW": np.asarray(w['res_W'], np.float32),
        "Wk": Wk_pack,
        "rows": np.tile(rows.reshape(1, 16 * H), (128, 1)),
        "eeW1": np.asarray(w['ee_W1'], np.float32).astype(BF16),
        "eeW2": np.asarray(w['ee_W2'], np.float32).astype(BF16),
        "eeb1": np.asarray(w['ee_b1'], np.float32).reshape(H, 1),
        "hW1": hW1_pack,
        "hW2": np.asarray(w['hW2'], np.float32),
        "ident_bf": np.eye(128, dtype=np.float32).astype(BF16),
        "ident_f": np.eye(128, dtype=np.float32),
    }
    in_maps = []
    for r in range(NCORES):
        m = dict(shared)
        m.update({
            "xT": p.xT[r], "idx": p.idx_all[r], "dstloc": p.dstloc_all[r],
            "eaT": p.eaT_all[r], "iota3": p.iota3, "gsel": p.gsel[r],
            "pmax_idx": p.pmax_idx[r],
        })
        in_maps.append(m)
    return in_maps


def kernel(**inputs):
    from concourse.bass_utils import run_bass_kernel_spmd
    p = make_plan(inputs['x'], inputs['edge_index'], inputs['batch'],
                  inputs['edge_attr'])
    nc = build_nc(p, inputs)
    in_maps = make_in_maps(p, inputs)
    res = run_bass_kernel_spmd(nc, in_maps, core_ids=list(range(NCORES)),
                               trace=False)
    return np.asarray(res.results[0]["out"], np.float32).copy()


# revision 57
# speedup vs baseline: 1.1138x; 1.0370x over previous
"""GCN classifier kernel for Trainium2, 8 NeuronCores.

Strategy: graph-aligned node sharding (64 graphs/core), padded to NPC nodes.
Edges bucketed by (dst-block-of-128, src-quarter) cells; per-edge messages
are fetched from an AllGather-replicated bf16 node table with SWDGE
dma_gather using merged 1024-idx s-major streams, then aggregated per dst
block with one-hot matmuls whose lhsT is an ew-weighted mask:

  agg[dst_blk] = sum_chunks maskW[:, :, c]^T @ msg[chunk]   (+ identity
  self-loop from the resident slab)

maskW is laid out [128 edge, 128 dst, C] with the chunk dim innermost so
both mask-build ops (is_equal vs dstloc, mult by ew) are packed-innermost
tensor_tensor ops that hit the DVE 2x perf mode; the matmul reads lhsT with
a strided AP. The ew multiply rides the aggregation matmul for free.

PSUM drains run on the scalar (ACT) engine (DVE PSUM reads are slow).
AllGather is split in two halves (half-major table layout) to overlap the
collective with the second half of each layer. Mean pooling is a one-hot
matmul with 1/count baked into the selector; max pooling keeps the gather
path. LayerNorm/residual/ReLU epilogues run per 128-node block on ACT/DVE.
"""

import sys
import types

sys.path.insert(0, "/opt/trn_rl_repo")

import numpy as np
import ml_dtypes

BF16 = ml_dtypes.bfloat16

# Shim antenv.axon_hooks (missing in this image) so trace=True can work.
try:
    import antenv.axon_hooks  # noqa: F401
except ImportError:
    try:
        from trn_agent_boot.trn_boot import _ntff_profile_via_ctypes
        _hook = _ntff_profile_via_ctypes('/opt/axon/libaxon_pjrt.so')
    except Exception:
        _hook = None
    _mod = types.ModuleType('antenv.axon_hooks')
    _mod.get_axon_ntff_profile_hook = lambda: _hook
    sys.modules['antenv.axon_hooks'] = _mod

import concourse.bacc as bacc
import concourse.mybir as mybir
import concourse.tile as tile
import concourse.bass_utils as bass_utils

# No bucket access in this container.
bass_utils.upload_artifacts = lambda tmpdir: tmpdir

F32 = mybir.dt.float32
BF = mybir.dt.bfloat16
I16 = mybir.dt.int16
AOP = mybir.AluOpType
ACTF = mybir.ActivationFunctionType
AXX = mybir.AxisListType.X

NCORES = 8
H = 128        # hidden channels
IN = 96        # in channels
ED = 8         # edge dim
NCLS = 100     # classes
L = 4          # layers
NGRAPH = 512   # graphs
GPC = NGRAPH // NCORES
EPS_LN = 1e-5
MAXG = 1024    # dma_gather num_idxs hard limit (2048 wedges the device)

import os
DBG_NOGATHER = os.environ.get("K_NOGATHER") == "1"
DBG_NOMAXPOOL = os.environ.get("K_NOMAXPOOL") == "1"
DBG_NOGMEAN = os.environ.get("K_NOGMEAN") == "1"
DBG_NOMASK = os.environ.get("K_NOMASK") == "1"


def _split_waits(nc, max_waits=1):
    """This container's walrus rejects >1 sync wait per instruction; move
    extra waits onto preceding NOPs on the same engine."""
    n = 0
    for f in nc.m.functions:
        for bb in f.blocks:
            new_list = []
            for ins in bb.instructions:
                si = ins.sync_info
                if si and si.on_wait and len(si.on_wait) > max_waits:
                    waits = list(si.on_wait)
                    extra, keep = waits[:-max_waits], waits[-max_waits:]
                    for i, w in enumerate(extra):
                        nop = mybir.InstNoOp(name=f"{ins.name}-ws{i}", ins=[], outs=[])
                        nop.engine = ins.engine
                        nop.sync_info = mybir.SyncInfo(on_wait=[w], on_update=[])
                        new_list.append(nop)
                        n += 1
                    si.on_wait = keep
                new_list.append(ins)
            bb.instructions[:] = new_list
    return n


def _fix_act_tables(nc, set_id=6):
    """All activation funcs used here live in act table 6
    (natural_log_exp_and_others); the greedy per-func chooser ping-pongs
    between tables 0/5 costing ~27us per reload. Unify and dedupe."""
    removed = 0
    for f in nc.m.functions:
        for bb in f.blocks:
            new_list = []
            loaded = False
            for ins in bb.instructions:
                if isinstance(ins, mybir.InstLoadActFuncSet):
                    ins.act_func_set_id = set_id
                    si = ins.sync_info
                    has_sync = si and (si.on_wait or si.on_update)
                    if loaded and not has_sync:
                        removed += 1
                        continue
                    if loaded and has_sync:
                        nop = mybir.InstNoOp(name=ins.name + "-actdedup", ins=[], outs=[])
                        nop.engine = ins.engine
                        nop.sync_info = si
                        new_list.append(nop)
                        removed += 1
                        continue
                    loaded = True
                new_list.append(ins)
            bb.instructions[:] = new_list
    return removed


def _ru(x, m):
    return (x + m - 1) // m * m


def _wrap_idxs(idx):
    """[n] int -> [128, n//16] int16 SBUF wrap (i -> partition i%16, col i//16),
    replicated over the 8 gpsimd cores."""
    n = len(idx)
    assert n % 16 == 0
    a = np.asarray(idx, np.int16).reshape(n // 16, 16).T.copy()
    return np.tile(a, (8, 1))


class Plan:
    pass


def make_plan(x, edge_index, batch, edge_attr):
    N = x.shape[0]
    E = edge_index.shape[1]
    p = Plan()
    p.N, p.E = N, E

    batch = np.asarray(batch, np.int64)
    src = np.asarray(edge_index[0], np.int64)
    dst = np.asarray(edge_index[1], np.int64)

    node_start = np.searchsorted(batch, np.arange(NGRAPH + 1))  # [513]
    core_start = node_start[::GPC][:NCORES].astype(np.int64)
    core_end = np.append(core_start[1:], N).astype(np.int64)
    core_cnt = core_end - core_start
    NPC = max(512, _ru(int(core_cnt.max()), 128))
    NBLK = NPC // 128
    NPAD = NCORES * NPC
    # Four gather windows (int16 idx => window rows <= 32767), uneven: the
    # last window is small so its AllGather (the only exposed one at layer
    # boundaries) is short.
    MAXWB = 32767 // (NCORES * 128)  # max window size in blocks (31)
    base = max(1, min(MAXWB, round(NBLK * 0.27)))
    wblk = [base, base, base, NBLK - 3 * base]
    assert 1 <= wblk[3] <= MAXWB, (NBLK, wblk)
    woff = np.concatenate([[0], np.cumsum(wblk)]).astype(np.int64)
    p.wblk, p.woff = wblk, woff
    p.NPC, p.NBLK, p.NPAD = NPC, NBLK, NPAD
    p.core_start, p.core_cnt = core_start, core_cnt

    owner = np.searchsorted(core_start, np.arange(N), side='right') - 1
    loc = np.arange(N) - core_start[owner]
    # window-major table row: table window w holds every core's local node
    # blocks [woff[w], woff[w+1]), and equals gather window w.
    blk_of = loc >> 7
    wsel = np.searchsorted(woff, blk_of, side='right') - 1
    table_row = (woff[wsel] * NCORES * 128 + owner * (woff[wsel + 1] - woff[wsel]) * 128
                 + (loc - woff[wsel] * 128))

    src_t_all = table_row[src]
    srange_all = wsel[src]
    d_owner = owner[dst]
    d_loc = loc[dst]
    blk_all = d_loc >> 7
    dloc_all = d_loc & 127
    cell_all = blk_all * 4 + srange_all  # b-major cell id
    NCELL = NBLK * 4

    order = np.lexsort((src_t_all, cell_all, d_owner))
    src_t = src_t_all[order]
    dloc = dloc_all[order]
    cell = cell_all[order]
    e_owner = d_owner[order]
    ea_perm = np.asarray(edge_attr, np.float32)[order]

    counts = np.zeros((NCORES, NCELL), np.int64)
    for r in range(NCORES):
        m = e_owner == r
        counts[r] = np.bincount(cell[m], minlength=NCELL)
    core_off = np.searchsorted(e_owner, np.arange(NCORES + 1))
    core_cell_off = np.zeros((NCORES, NCELL + 1), np.int64)
    for r in range(NCORES):
        core_cell_off[r, 0] = core_off[r]
        core_cell_off[r, 1:] = np.cumsum(counts[r]) + core_off[r]

    cnum = _ru(counts.max(axis=0), 128)     # padded idx count per cell
    Cg = cnum // 128                        # chunks per cell
    p.Cg = Cg

    # b-major chunk columns
    cell_dl = np.concatenate([[0], np.cumsum(Cg)]).astype(np.int64)
    NDL = int(cell_dl[-1])
    p.NDL = NDL
    C_b = np.array([int(cell_dl[b * 4 + 4] - cell_dl[b * 4]) for b in range(NBLK)])
    p.C_b = C_b
    p.Cmax = int(C_b.max())
    assert p.Cmax >= 2

    # s-major gather streams: stream s = concat over b of cell (b, s) chunks
    stream_cells = [[b * 4 + s for b in range(NBLK)] for s in range(4)]
    stream_len = [int(sum(Cg[c] for c in cs)) for cs in stream_cells]  # chunks
    G_s = [(sl * 128 + MAXG - 1) // MAXG for sl in stream_len]
    p.G_s = G_s
    gbase = np.concatenate([[0], np.cumsum(G_s)]).astype(np.int64)
    p.NGATH = int(gbase[-1])

    # chunk (b-major col j) -> (gather g, slot) and per-(b,s) gather needs
    chunk_gather = np.zeros(NDL, np.int64)
    chunk_slot = np.zeros(NDL, np.int64)
    need_g = np.zeros((NBLK, 4), np.int64)   # gathers of stream s needed
    pos_s = [0, 0, 0, 0]
    for b in range(NBLK):
        for s in range(4):
            c = b * 4 + s
            for k in range(Cg[c]):
                j = cell_dl[c] + k
                pos = pos_s[s]
                chunk_gather[j] = gbase[s] + pos // 8
                chunk_slot[j] = pos % 8
                pos_s[s] += 1
            need_g[b, s] = (pos_s[s] + 7) // 8  # ceil chunks/8 so far
    p.chunk_gather, p.chunk_slot, p.need_g = chunk_gather, chunk_slot, need_g

    # fill per-core data
    dstloc_f = np.full((NCORES, 128, NDL), 255.0, np.float32)
    p.eaT_all = np.zeros((NCORES, 8, NDL * 128), BF16)
    idx_stream = np.zeros((NCORES, 4, max(G_s) * MAXG), np.int64)
    for r in range(NCORES):
        spos = [0, 0, 0, 0]
        for b in range(NBLK):
            for s in range(4):
                c = b * 4 + s
                if Cg[c] == 0:
                    continue
                a0 = core_cell_off[r, c]
                a1 = core_cell_off[r, c + 1]
                n_real = int(a1 - a0)
                num = int(cnum[c])
                # idxs for this cell (pad slots -> 0)
                iv = np.zeros(num, np.int64)
                if n_real:
                    iv[:n_real] = src_t[a0:a1] - int(woff[s]) * NCORES * 128
                idx_stream[r, s, spos[s]:spos[s] + num] = iv
                spos[s] += num
                # dstloc cols (b-major)
                dl = np.full(num, 255.0, np.float32)
                if n_real:
                    dl[:n_real] = dloc[a0:a1].astype(np.float32)
                dstloc_f[r, :, cell_dl[c]:cell_dl[c + 1]] = \
                    dl.reshape(Cg[c], 128).T
                # edge attrs (b-major)
                if n_real:
                    ea = np.zeros((num, ED), np.float32)
                    ea[:n_real] = ea_perm[a0:a1]
                    p.eaT_all[r, :, cell_dl[c] * 128:cell_dl[c + 1] * 128] = \
                        ea.T.astype(BF16)
    p.dstloc_all = dstloc_f.astype(BF16)

    # wrap idx streams into gather-major int16 [128, NGATH*64]
    p.idx_all = np.zeros((NCORES, 128, p.NGATH * (MAXG // 16)), np.int16)
    for r in range(NCORES):
        for s in range(4):
            for g in range(G_s[s]):
                iv = idx_stream[r, s, g * MAXG:(g + 1) * MAXG]
                gg = int(gbase[s]) + g
                p.idx_all[r, :, gg * 64:(gg + 1) * 64] = _wrap_idxs(iv)

    # x slab, transposed [96, NPC] per core
    p.xT = np.zeros((NCORES, IN, NPC), np.float32)
    xf = np.asarray(x, np.float32)
    for r in range(NCORES):
        p.xT[r, :, :core_cnt[r]] = xf[core_start[r]:core_end[r]].T

    # mean pooling selector: gsel[node p of block b, g] = 1/count(g)
    gcnt = (node_start[1:] - node_start[:-1]).astype(np.int64)
    p.gsel = np.zeros((NCORES, 128, NBLK * GPC), np.float32)
    for r in range(NCORES):
        for lid in range(int(core_cnt[r])):
            g = int(batch[core_start[r] + lid])
            jl = g - r * GPC
            b, pp = lid >> 7, lid & 127
            p.gsel[r, pp, b * GPC + jl] = 1.0 / max(int(gcnt[g]), 1)
    p.gsel = p.gsel.astype(BF16)

    # max pooling: gather idx per graph padded to MAXN (repeat first node)
    MAXN = max(128, _ru(int(gcnt.max()), 128))
    assert MAXN <= MAXG
    p.MAXN = MAXN
    ZROW = NPC
    p.pmax_idx = np.zeros((NCORES, 128, GPC * MAXN // 16), np.int16)
    for r in range(NCORES):
        mi = []
        for j in range(GPC):
            gid = r * GPC + j
            a = int(node_start[gid] - core_start[r])
            n = int(gcnt[gid])
            ids = np.arange(a, a + n)
            pad = MAXN - n
            mi.append(np.concatenate([ids, np.full(pad, ids[0] if n else ZROW)]))
        p.pmax_idx[r] = _wrap_idxs(np.concatenate(mi))
    # block (uniform across cores) after which max-gather group j0 can fire
    GPCALL = max(1, MAXG // MAXN)
    p.pool_grp_blk = np.zeros(GPC, np.int64)
    for j0 in range(0, GPC, GPCALL):
        j1 = min(j0 + GPCALL, GPC)
        endmax = 0
        for r in range(NCORES):
            e = int(node_start[min(r * GPC + j1, NGRAPH)] - core_start[r])
            endmax = max(endmax, (e + 127) // 128)
        p.pool_grp_blk[j0] = min(endmax, NBLK) - 1

    # iota3 [128, 128, Cmax] bf16: value d at (p, d, c)
    p.iota3 = np.tile(
        np.arange(128, dtype=np.float32)[None, :, None],
        (128, 1, p.Cmax)).reshape(128, 128 * p.Cmax).astype(BF16)
    return p


def build_nc(p, w):
    nc = bacc.Bacc("TRN2", num_devices=NCORES, detect_race_conditions=False,
                   num_swdge_queues=4)
    NPC, NBLK, NPAD = p.NPC, p.NBLK, p.NPAD
    wblk, woff = p.wblk, p.woff
    NDL, Cmax, C_b, Cg = p.NDL, p.Cmax, p.C_b, p.Cg
    cell_dl = np.concatenate([[0], np.cumsum(Cg)]).astype(np.int64)

    # ---- I/O ----
    t_xT = nc.dram_tensor("xT", [IN, NPC], F32, kind="ExternalInput")
    t_idx = nc.dram_tensor("idx", [128, p.NGATH * 64], I16, kind="ExternalInput")
    t_dstloc = nc.dram_tensor("dstloc", [128, NDL], BF, kind="ExternalInput")
    t_eaT = nc.dram_tensor("eaT", [8, NDL * 128], BF, kind="ExternalInput")
    t_iota3 = nc.dram_tensor("iota3", [128, 128 * Cmax], BF, kind="ExternalInput")
    t_gsel = nc.dram_tensor("gsel", [128, NBLK * GPC], BF, kind="ExternalInput")
    t_pmax_idx = nc.dram_tensor("pmax_idx", [128, GPC * p.MAXN // 16], I16,
                                kind="ExternalInput")
    t_W0 = nc.dram_tensor("W0", [IN, H], F32, kind="ExternalInput")
    t_resW = nc.dram_tensor("resW", [IN, H], F32, kind="ExternalInput")
    t_Wk = nc.dram_tensor("Wk", [H, 3 * H], F32, kind="ExternalInput")
    t_rows = nc.dram_tensor("rows", [128, 16 * H], F32, kind="ExternalInput")
    t_eeW1 = nc.dram_tensor("eeW1", [ED, H], BF, kind="ExternalInput")
    t_eeW2 = nc.dram_tensor("eeW2", [H, 1], BF, kind="ExternalInput")
    t_eeb1 = nc.dram_tensor("eeb1", [H, 1], F32, kind="ExternalInput")
    t_hW1 = nc.dram_tensor("hW1", [H, 2 * H], F32, kind="ExternalInput")
    t_hW2 = nc.dram_tensor("hW2", [H, NCLS], F32, kind="ExternalInput")
    t_ident_bf = nc.dram_tensor("ident_bf", [128, 128], BF, kind="ExternalInput")
    t_ident_f = nc.dram_tensor("ident_f", [128, 128], F32, kind="ExternalInput")
    t_out = nc.dram_tensor("out", [NGRAPH, NCLS], F32, kind="ExternalOutput")

    ee_b2 = float(np.asarray(w['ee_b2']).reshape(-1)[0])

    gq_counter = [0]

    def next_q():
        q = gq_counter[0] % 4
        gq_counter[0] += 1
        return q

    with tile.TileContext(nc) as tc:
        with (
            tc.tile_pool(name="const", bufs=1) as cp,
            tc.tile_pool(name="dram", bufs=1, space="DRAM") as dp,
            tc.tile_pool(name="ea", bufs=2) as eap,
            tc.tile_pool(name="eh", bufs=2) as ehp,
            tc.tile_pool(name="msg", bufs=13) as msgp,
            tc.tile_pool(name="mask", bufs=3) as maskp,
            tc.tile_pool(name="blk", bufs=3) as bp,
            tc.tile_pool(name="pag", bufs=3, space="PSUM") as pag,
            tc.tile_pool(name="pgm", bufs=1, space="PSUM") as pgm,
            tc.tile_pool(name="pmm", bufs=2, space="PSUM") as pmm,
            tc.tile_pool(name="psm", bufs=2, space="PSUM") as psm,
        ):
            # ---------- resident tiles ----------
            def load_const(t, shape, dtype, tag):
                tl = cp.tile(shape, dtype, tag=tag)
                nc.sync.dma_start(tl[:], t[:])
                return tl

            idx_sb = load_const(t_idx, [128, p.NGATH * 64], I16, "idx_sb")
            dstloc = load_const(t_dstloc, [128, NDL], BF, "dstloc")
            iota3 = load_const(t_iota3, [128, 128 * Cmax], BF, "iota3")
            gsel = load_const(t_gsel, [128, NBLK * GPC], BF, "gsel")
            W0 = load_const(t_W0, [IN, H], F32, "W0")
            resW = load_const(t_resW, [IN, H], F32, "resW")
            Wk = load_const(t_Wk, [H, 3 * H], F32, "Wk")
            rows = load_const(t_rows, [128, 16 * H], F32, "rows")
            eeW1 = load_const(t_eeW1, [ED, H], BF, "eeW1")
            eeW2 = load_const(t_eeW2, [H, 1], BF, "eeW2")
            eeb1 = load_const(t_eeb1, [H, 1], F32, "eeb1")
            hW1 = load_const(t_hW1, [H, 2 * H], F32, "hW1")
            hW2 = load_const(t_hW2, [H, NCLS], F32, "hW2")
            ident_bf = load_const(t_ident_bf, [128, 128], BF, "ident_bf")
            ident_f = load_const(t_ident_f, [128, 128], F32, "ident_f")
            pmax_idx = load_const(t_pmax_idx, [128, GPC * p.MAXN // 16],
                                  I16, "pmax_idx")

            b2col = cp.tile([128, 1], F32, tag="b2col")
            nc.vector.memset(b2col[:], ee_b2)
            epscol = cp.tile([128, 1], F32, tag="epscol")
            nc.vector.memset(epscol[:], EPS_LN)
            onescol = cp.tile([128, 1], BF, tag="onescol")
            nc.vector.memset(onescol[:], 1.0)
            ew = cp.tile([128, NDL], BF, tag="ew")
            slab = cp.tile([128, NBLK * 128], BF, tag="slab")
            dis = cp.tile([128, NBLK], F32, tag="dis")
            gmaxT = cp.tile([128, GPC], F32, tag="gmaxT")
            nc.vector.memset(gmaxT[:], 0.0)

            # ---------- DRAM scratch ----------
            tables = []  # [layer][window]
            for _k in range(L):
                tables.append([
                    dp.tile([wblk[_q] * 128 * NCORES, H], BF,
                            addr_space="Shared",
                            tag=f"table{_k}q{_q}", name=f"table{_k}q{_q}")
                    for _q in range(4)])
            slab_hbm = [dp.tile([wblk[_q] * 128, H], BF, tag=f"slabq{_q}",
                                name=f"slabq{_q}")
                        for _q in range(4)]
            h_hbm_a = dp.tile([NPC, H], F32, tag="h_hbm_a")
            h_hbm_b = dp.tile([NPC, H], F32, tag="h_hbm_b")
            h_hbm = [h_hbm_a, h_hbm_b]
            hsum_hbm = dp.tile([NPC, H], F32)
            res0_hbm = dp.tile([NPC, H], F32)
            pool_tab = dp.tile([NPC + 16, H], BF)
            opart = dp.tile([GPC, NCLS], F32)
            gout = dp.tile([NGRAPH, NCLS], F32, addr_space="Shared")

            def maybe_store_slab(b):
                if b + 1 in [int(x) for x in woff[1:]]:
                    q = [int(x) for x in woff[1:]].index(b + 1)
                    nc.sync.dma_start(
                        slab_hbm[q][:].rearrange("(b q) f -> q b f", q=128),
                        slab[:, int(woff[q]) * 128:int(woff[q + 1]) * 128]
                        .rearrange("p (b f) -> p b f", f=H))

            def emit_ag(k, qq):
                nc.gpsimd.collective_compute(
                    "AllGather", AOP.bypass,
                    replica_groups=[list(range(NCORES))],
                    ins=[slab_hbm[qq][:].opt()],
                    outs=[tables[k][qq][:].opt()])

            def emit_ags(k):
                for qq in range(4):
                    emit_ag(k, qq)

            def gather_in_ap(k, s):
                return tables[k][s][:, :]

            def build_mask(mk, b):
                dl0 = int(cell_dl[b * 4])
                cb = int(C_b[b])
                if cb == 0:
                    return None
                mk3 = mk[:, :128 * cb].rearrange("p (d c) -> p d c", c=cb)
                nc.vector.tensor_tensor(
                    out=mk3,
                    in0=iota3[:, :].rearrange("p (d c) -> p d c", c=Cmax)[:, :, :cb],
                    in1=dstloc[:, dl0:dl0 + cb].unsqueeze(1)
                        .to_broadcast([128, 128, cb]),
                    op=AOP.is_equal)
                nc.vector.tensor_tensor(
                    out=mk3, in0=mk3,
                    in1=ew[:, dl0:dl0 + cb].unsqueeze(1)
                        .to_broadcast([128, 128, cb]),
                    op=AOP.mult)
                return mk3

            # =============== preamble: edge MLP + degree + hws0 ===============
            for b in range(NBLK):
                dl0 = int(cell_dl[b * 4])
                cb = int(C_b[b])
                cols = cb * 128
                if cb == 0:
                    nc.vector.memset(dis[:, b:b + 1], 1.0)
                    bs = slice(b * 128, (b + 1) * 128)
                    xtb = bp.tile([IN, 128], F32, tag="xtb")
                    nc.sync.dma_start(xtb[:], t_xT[:, bs])
                    hw_ps = pmm.tile([128, 512], F32, tag="mm")
                    nc.tensor.matmul(hw_ps[:, :H], lhsT=xtb[:, :],
                                     rhs=W0[:, :], start=True, stop=True)
                    nc.scalar.mul(slab[:, bs], hw_ps[:, :H], dis[:, b:b + 1])
                    rs_ps = pmm.tile([128, 512], F32, tag="mm")
                    nc.tensor.matmul(rs_ps[:, :H], lhsT=xtb[:, :],
                                     rhs=resW[:, :], start=True, stop=True)
                    r0 = bp.tile([128, H], F32, tag="r0")
                    nc.vector.tensor_tensor(
                        out=r0[:], in0=rs_ps[:, :H],
                        in1=rows[:, 12 * H:13 * H], op=AOP.add)
                    nc.sync.dma_start(res0_hbm[bs, :], r0[:])
                    maybe_store_slab(b)
                    continue
                # --- edge MLP for this block's chunks ---
                ea_t = eap.tile([8, Cmax * 128], BF, tag="ea")
                nc.sync.dma_start(ea_t[:, :cols],
                                  t_eaT[:, dl0 * 128:dl0 * 128 + cols])
                eh = ehp.tile([128, 128 * Cmax], BF, tag="eh")
                for c0 in range(0, cols, 512):
                    c1 = min(c0 + 512, cols)
                    eh_ps = pmm.tile([128, 512], F32, tag="mm")
                    nc.tensor.matmul(eh_ps[:, :c1 - c0], lhsT=eeW1[:, :],
                                     rhs=ea_t[:, c0:c1], start=True, stop=True)
                    nc.scalar.activation(eh[:, c0:c1], eh_ps[:, :c1 - c0],
                                         ACTF.Relu, bias=eeb1[:, :], scale=1.0)
                ewz_ps = psm.tile([128, Cmax], F32, tag="sm")
                for c in range(cb):
                    nc.tensor.matmul(ewz_ps[:, c:c + 1],
                                     lhsT=eh[:, c * 128:(c + 1) * 128],
                                     rhs=eeW2[:, :], start=True, stop=True,
                                     skip_group_check=True)
                # softplus(z + b2) + 1e-4, into block-local ewb + persistent ew
                ezb = bp.tile([128, Cmax], F32, tag="ezb")
                nc.scalar.activation(ezb[:, :cb], ewz_ps[:, :cb], ACTF.Exp,
                                     bias=b2col[:, :], scale=1.0)
                ewb = bp.tile([128, Cmax], BF, tag="ewb")
                nc.scalar.activation(ewb[:, :cb], ezb[:, :cb], ACTF.Ln,
                                     bias=1.0, scale=1.0)
                nc.vector.tensor_scalar(ewb[:, :cb], ewb[:, :cb], 1e-4,
                                        None, AOP.add)
                nc.scalar.activation(ew[:, dl0:dl0 + cb], ewb[:, :cb],
                                     ACTF.Identity, bias=0.0, scale=1.0)

                # --- weighted mask (block-local ew) + degree ---
                mk = maskp.tile([128, 128 * Cmax], BF, tag="mask")
                mk3 = mk[:, :128 * cb].rearrange("p (d c) -> p d c", c=cb)
                nc.vector.tensor_tensor(
                    out=mk3,
                    in0=iota3[:, :].rearrange("p (d c) -> p d c", c=Cmax)[:, :, :cb],
                    in1=dstloc[:, dl0:dl0 + cb].unsqueeze(1)
                        .to_broadcast([128, 128, cb]),
                    op=AOP.is_equal)
                nc.vector.tensor_tensor(
                    out=mk3, in0=mk3,
                    in1=ewb[:, :cb].unsqueeze(1).to_broadcast([128, 128, cb]),
                    op=AOP.mult)
                deg_ps = psm.tile([128, Cmax], F32, tag="sm")
                for c in range(cb):
                    nc.tensor.matmul(
                        deg_ps[:, :1], lhsT=mk3[:, :, c],
                        rhs=onescol[:, :],
                        start=(c == 0), stop=(c == cb - 1),
                        skip_group_check=True)
                lntmp = bp.tile([128, 1], F32, tag="lntmp")
                nc.scalar.activation(lntmp[:], deg_ps[:, :1], ACTF.Ln,
                                     bias=1.0, scale=1.0)
                nc.scalar.activation(dis[:, b:b + 1], lntmp[:], ACTF.Exp,
                                     bias=0.0, scale=-0.5)

                # --- hws0 / res0 ---
                bs = slice(b * 128, (b + 1) * 128)
                xtb = bp.tile([IN, 128], F32, tag="xtb")
                nc.sync.dma_start(xtb[:], t_xT[:, bs])
                hw_ps = pmm.tile([128, 512], F32, tag="mm")
                nc.tensor.matmul(hw_ps[:, :H], lhsT=xtb[:, :],
                                 rhs=W0[:, :], start=True, stop=True)
                nc.scalar.mul(slab[:, bs], hw_ps[:, :H], dis[:, b:b + 1])
                rs_ps = pmm.tile([128, 512], F32, tag="mm")
                nc.tensor.matmul(rs_ps[:, :H], lhsT=xtb[:, :],
                                 rhs=resW[:, :], start=True, stop=True)
                r0 = bp.tile([128, H], F32, tag="r0")
                nc.vector.tensor_tensor(
                    out=r0[:], in0=rs_ps[:, :H],
                    in1=rows[:, 12 * H:13 * H], op=AOP.add)
                nc.sync.dma_start(res0_hbm[bs, :], r0[:])
                maybe_store_slab(b)

            # zero row for max-pool pads (pool_tab tail), before layer 3 uses it
            zr = bp.tile([16, H], BF, tag="zr")
            nc.vector.memset(zr[:], 0.0)
            nc.sync.dma_start(pool_tab[NPC:NPC + 16, :], zr[:])

            # =============== layers ===============
            GPCALL = max(1, MAXG // p.MAXN)
            CPG = p.MAXN // 128          # chunks per graph

            def maxpool_group(j0):
                j1 = min(j0 + GPCALL, GPC)
                nidx = (j1 - j0) * p.MAXN
                pg = msgp.tile([128, 8, 128], BF, tag="poolmax")
                nc.gpsimd.dma_gather(
                    out_ap=pg[:, :nidx // 128, :],
                    in_ap=pool_tab[:],
                    idxs_ap=pmax_idx[:, j0 * p.MAXN // 16:
                                     j0 * p.MAXN // 16 + nidx // 16],
                    num_idxs=nidx, num_idxs_reg=nidx,
                    elem_size=H, queue_num=next_q())
                for j in range(j0, j1):
                    tp = psm.tile([128, 128 * CPG], BF, tag="sm")
                    for cc in range(CPG):
                        ch = pg[:, (j - j0) * CPG + cc, :]
                        nc.tensor.transpose(tp[:, cc * 128:(cc + 1) * 128],
                                            ch, ident_bf[:, :])
                    nc.vector.reduce_max(gmaxT[:, j:j + 1], tp[:], axis=AXX)

            for k in range(L):
                gbase = np.concatenate([[0], np.cumsum(p.G_s)]).astype(int)
                issued = [0, 0, 0, 0]
                gtiles = {}

                def issue_gather(s):
                    gg = int(gbase[s]) + issued[s]
                    msg = msgp.tile([128, 8, 128], BF, tag="msg")
                    nc.gpsimd.dma_gather(
                        out_ap=msg[:, :, :],
                        in_ap=gather_in_ap(k, s),
                        idxs_ap=idx_sb[:, gg * 64:(gg + 1) * 64],
                        num_idxs=MAXG, num_idxs_reg=MAXG,
                        elem_size=H, queue_num=next_q())
                    gtiles[gg] = msg
                    issued[s] += 1

                emit_ags(k)

                for b in range(NBLK):
                    # ensure gathers covering this block are issued
                    for s in range(4 if not DBG_NOGATHER else 0):
                        while issued[s] < int(p.need_g[b, s]):
                            issue_gather(s)

                    dl0 = int(cell_dl[b * 4])
                    cb = int(C_b[b])
                    if DBG_NOGATHER or DBG_NOMASK:
                        cb = 0
                    if cb:
                        mk = maskp.tile([128, 128 * Cmax], BF, tag="mask")
                        mk3 = build_mask(mk, b)

                    agg = pag.tile([128, H], F32, tag="agg")
                    for ci in range(cb):
                        j = dl0 + ci
                        msg = gtiles[int(p.chunk_gather[j])]
                        nc.tensor.matmul(
                            agg[:], lhsT=mk3[:, :, ci],
                            rhs=msg[:, int(p.chunk_slot[j]), :],
                            start=(ci == 0), stop=False,
                            skip_group_check=True)
                    bs = slice(b * 128, (b + 1) * 128)
                    nc.tensor.matmul(agg[:], lhsT=ident_bf[:, :],
                                     rhs=slab[:, bs],
                                     start=(cb == 0), stop=True,
                                     skip_group_check=True)

                    # ---------- epilogue ----------
                    u = bp.tile([128, H], F32, tag="u")
                    nc.scalar.mul(u[:], agg[:], dis[:, b:b + 1])
                    nc.vector.tensor_tensor(
                        out=u[:], in0=u[:],
                        in1=rows[:, k * H:(k + 1) * H], op=AOP.add)
                    mu = bp.tile([128, 1], F32, tag="mu")
                    nc.vector.reduce_sum(mu[:], u[:], axis=AXX)
                    nc.vector.tensor_scalar(mu[:], mu[:], -1.0 / H, None, AOP.mult)
                    xc = bp.tile([128, H], F32, tag="xc")
                    nc.scalar.activation(xc[:], u[:], ACTF.Identity,
                                         bias=mu[:, :], scale=1.0)
                    sq = bp.tile([128, H], F32, tag="sq")
                    var = bp.tile([128, 1], F32, tag="var")
                    nc.scalar.activation(sq[:], xc[:], ACTF.Square,
                                         bias=0.0, scale=1.0, accum_out=var[:])
                    lnv = bp.tile([128, 1], F32, tag="lnv")
                    nc.scalar.activation(lnv[:], var[:], ACTF.Ln,
                                         bias=epscol[:, :], scale=1.0 / H)
                    inv = bp.tile([128, 1], F32, tag="inv")
                    nc.scalar.activation(inv[:], lnv[:], ACTF.Exp,
                                         bias=0.0, scale=-0.5)
                    y = bp.tile([128, H], F32, tag="y")
                    nc.scalar.mul(y[:], xc[:], inv[:, :])
                    nc.vector.tensor_tensor(
                        out=y[:], in0=y[:],
                        in1=rows[:, (4 + k) * H:(5 + k) * H], op=AOP.mult)
                    nc.vector.tensor_tensor(
                        out=y[:], in0=y[:],
                        in1=rows[:, (8 + k) * H:(9 + k) * H], op=AOP.add)
                    res = bp.tile([128, H], F32, tag="res")
                    if k == 0:
                        nc.sync.dma_start(res[:], res0_hbm[bs, :])
                    else:
                        nc.sync.dma_start(res[:], h_hbm[(k - 1) % 2][bs, :])
                    h = bp.tile([128, H], F32, tag="h")
                    nc.vector.tensor_tensor(out=h[:], in0=y[:], in1=res[:],
                                            op=AOP.add)
                    nc.scalar.activation(h[:], h[:], ACTF.Relu, bias=0.0, scale=1.0)
                    # hsum accumulation in HBM
                    if k == 0:
                        nc.sync.dma_start(hsum_hbm[bs, :], h[:])
                    else:
                        hs = bp.tile([128, H], F32, tag="hs")
                        nc.sync.dma_start(hs[:], hsum_hbm[bs, :])
                        nc.vector.tensor_tensor(out=hs[:], in0=hs[:], in1=h[:],
                                                op=AOP.add)
                        if k < L - 1:
                            nc.sync.dma_start(hsum_hbm[bs, :], hs[:])
                        else:
                            xm = bp.tile([128, H], BF, tag="xm")
                            nc.scalar.activation(xm[:], hs[:], ACTF.Identity,
                                                 bias=0.0, scale=0.25)
                            nc.sync.dma_start(pool_tab[bs, :], xm[:])
                            # mean pooling via selector matmul (PSUM chain)
                            if b == 0:
                                gmean_ps = pgm.tile([GPC, 512], F32, tag="gmean")
                                if DBG_NOGMEAN:
                                    nc.vector.memset(gmean_ps[:, :H], 0.0)
                            if not DBG_NOGMEAN:
                                nc.tensor.matmul(
                                    gmean_ps[:, :H],
                                    lhsT=gsel[:, b * GPC:(b + 1) * GPC],
                                    rhs=xm[:], start=(b == 0), stop=(b == NBLK - 1),
                                    skip_group_check=True)
                            # interleave ready max-pool gather groups
                            if not DBG_NOMAXPOOL:
                                for j0 in range(0, GPC, GPCALL):
                                    if int(p.pool_grp_blk[j0]) == b:
                                        maxpool_group(j0)
                    if k < L - 1:
                        nc.sync.dma_start(h_hbm[k % 2][bs, :], h[:])
                        hT_ps = pmm.tile([128, 512], F32, tag="mm")
                        nc.tensor.transpose(hT_ps[:, :H], h[:], ident_f[:, :])
                        hT = bp.tile([128, H], F32, tag="hT")
                        nc.scalar.activation(hT[:], hT_ps[:, :H], ACTF.Identity,
                                             bias=0.0, scale=1.0)
                        hw_ps = pmm.tile([128, 512], F32, tag="mm")
                        nc.tensor.matmul(hw_ps[:, :H], lhsT=hT[:],
                                         rhs=Wk[:, k * H:(k + 1) * H],
                                         start=True, stop=True)
                        nc.scalar.mul(slab[:, bs], hw_ps[:, :H], dis[:, b:b + 1])
                        maybe_store_slab(b)

            # =============== per-core head on own 64 graphs ===============
            # gmean: PSUM [GPC, H] -> SBUF, transpose to [128 h, GPC]
            gmean_sb = bp.tile([GPC, H], F32, tag="gmean_sb")
            nc.scalar.activation(gmean_sb[:], gmean_ps[:, :H], ACTF.Identity,
                                 bias=0.0, scale=1.0)
            gmT_ps = psm.tile([128, 128], F32, tag="sm")
            nc.tensor.transpose(gmT_ps[:, :GPC], gmean_sb[:],
                                ident_f[:GPC, :GPC])
            gmT = bp.tile([128, GPC], F32, tag="gmT")
            nc.scalar.activation(gmT[:], gmT_ps[:, :GPC], ACTF.Identity,
                                 bias=0.0, scale=1.0)
            # h1 [GPC, H] = gmean @ hW1[:H] + gmax @ hW1[H:]; gmaxT is
            # already [128 h, GPC] = the needed lhsT
            h1_ps = pmm.tile([128, 512], F32, tag="mm")
            nc.tensor.matmul(h1_ps[:GPC, :H], lhsT=gmT[:, :GPC],
                             rhs=hW1[:, 0:H], start=True, stop=False,
                             skip_group_check=True)
            nc.tensor.matmul(h1_ps[:GPC, :H], lhsT=gmaxT[:, :GPC],
                             rhs=hW1[:, H:2 * H], start=False, stop=True,
                             skip_group_check=True)
            h1 = bp.tile([GPC, H], F32, tag="h1")
            nc.vector.tensor_tensor(
                out=h1[:], in0=h1_ps[:GPC, :H],
                in1=rows[:GPC, 13 * H:14 * H], op=AOP.add)
            nc.vector.tensor_scalar(h1[:], h1[:], 0.0, None, AOP.max)
            h1T_ps = psm.tile([128, 128], F32, tag="sm")
            nc.tensor.transpose(h1T_ps[:, :GPC], h1[:], ident_f[:GPC, :GPC])
            h1T = bp.tile([128, GPC], F32, tag="h1T")
            nc.scalar.activation(h1T[:], h1T_ps[:, :GPC], ACTF.Identity,
                                 bias=0.0, scale=1.0)
            o_ps = pmm.tile([128, 512], F32, tag="mm")
            nc.tensor.matmul(o_ps[:GPC, :NCLS], lhsT=h1T[:, :GPC],
                             rhs=hW2[:, :], start=True, stop=True,
                             skip_group_check=True)
            o = bp.tile([GPC, NCLS], F32, tag="o")
            nc.vector.tensor_tensor(
                out=o[:], in0=o_ps[:GPC, :NCLS],
                in1=rows[:GPC, 14 * H:14 * H + NCLS], op=AOP.add)
            nc.sync.dma_start(opart[:, :], o[:])
            nc.gpsimd.collective_compute(
                "AllGather", AOP.bypass,
                replica_groups=[list(range(NCORES))],
                ins=[opart[:].opt()], outs=[gout[:].opt()])
            nc.sync.dma_start(t_out[:, :], gout[:, :])

    nc.compile()
    _fix_act_tables(nc)
    _split_waits(nc)
    return nc


def make_in_maps(p, w):
    rows = np.zeros((16, H), np.float32)  # replicated below
    for i in range(4):
        rows[i] = np.asarray(w[f'cb{i}'], np.float32)
        rows[4 + i] = np.asarray(w[f'g{i}'], np.float32)
        rows[8 + i] = np.asarray(w[f'be{i}'], np.float32)
    rows[12] = np.asarray(w['res_b'], np.float32)
    rows[13] = np.asarray(w['hb1'], np.float32)
    rows[14, :NCLS] = np.asarray(w['hb2'], np.float32)
    hW1 = np.asarray(w['hW1'], np.float32)          # [256, 128]
    hW1_pack = np.concatenate([hW1[:H, :], hW1[H:, :]], axis=1)  # [128, 256]
    Wk_pack = np.concatenate(
        [np.asarray(w[f'W{i}'], np.float32) for i in (1, 2, 3)], axis=1)
    shared = {
        "W0": np.asarray(w['W0'], np.float32),
        "resW": np.asarray(w['res_W'], np.float32),
        "Wk": Wk_pack,
        "rows": np.tile(rows.reshape(1, 16 * H), (128, 1)),
        "eeW1": np.asarray(w['ee_W1'], np.float32).astype(BF16),
        "eeW2": np.asarray(w['ee_W2'], np.float32).astype(BF16),
        "eeb1": np.asarray(w['ee_b1'], np.float32).reshape(H, 1),
        "hW1": hW1_pack,
        "hW2": np.asarray(w['hW2'], np.float32),
        "ident_bf": np.eye(128, dtype=np.float32).astype(BF16),
        "ident_f": np.eye(128, dtype=np.float32),
    }
    in_maps = []
    for r in range(NCORES):
        m = dict(shared)
        m.update({
            "xT": p.xT[r], "idx": p.idx_all[r], "dstloc": p.dstloc_all[r],
            "eaT": p.eaT_all[r], "iota3": p.iota3, "gsel": p.gsel[r],
            "pmax_idx": p.pmax_idx[r],
        })
        in_maps.append(m)
    return in_maps


def kernel(**inputs):
    from concourse.bass_utils import run_bass_kernel_spmd
    p = make_plan(inputs['x'], inputs['edge_index'], inputs['batch'],
                  inputs['edge_attr'])
    nc = build_nc(p, inputs)
    in_maps = make_in_maps(p, inputs)
    res = run_bass_kernel_spmd(nc, in_maps, core_ids=list(range(NCORES)),
                               trace=False)
    return np.asarray(res.results[0]["out"], np.float32).copy()
